# revision 31
# baseline (speedup 1.0000x reference)
"""Trainium2 Bass kernel for nn_DocREModel (8-core SPMD).

Sharding: data-parallel over the 4 documents x 2 halves = 8 cores.
Each doc's two cores duplicate the (cheap) graph phase, then split the
expensive conv reasoning stack SPATIALLY: core A computes output rows
0..10 of the 22x22 relation map, core B rows 11..21.  B works on a
row-FLIPPED frame with row-flipped conv taps so both cores run the
identical instruction stream (SPMD) -- only the data differs.  Pairs
are split by head entity (h<=10 vs h>=11), padded to PH2=288 columns.

e_ctx is reassociated:  ean @ (X @ W + b) == (ean @ X) @ W + rowsum(ean)*b,
which removes the full [1024,512] seq transform.

All floating-point arithmetic runs on device.  Host does only
index-driven data movement: batch slicing, transposes, row gathers at
integer indices, one-hot/selector construction, and layout packing.

DMA discipline: every sizable tensor is packed host-side as [128, W]
with per-partition-contiguous rows so each tensor is ONE dma_start
(the SP sequencer pays ~565ns per dma_start; the baseline's 334 DMAs
serialized 205us of sync time).
"""

import numpy as np
from contextlib import ExitStack

import concourse.bass as bass
import concourse.bacc as bacc
import concourse.tile as tile
import concourse.mybir as mybir
from concourse.bass_utils import run_bass_kernel_spmd

import ml_dtypes

FP32 = mybir.dt.float32
BF16 = mybir.dt.bfloat16

SEQ_DT = BF16
CONV_DT = BF16
PAIR_DT = BF16
GRAPH_DT = BF16

_NPDT = {FP32: np.float32, BF16: ml_dtypes.bfloat16}

B, C, H, NH = 4, 1024, 768, 12
E, M, L, LS = 22, 3, 30, 16
NN, EMB = 118, 512
P = 462
IC = 256
S = 22              # spatial side of relation map
PW = 26             # padded column width (2+22+2)
R1, R2, R3 = 15, 13, 11   # conv1/2/3 computed rows (half + halo)
BIN, B1R, B2R = 21, 19, 17  # padded row counts of conv input/1/2 buffers
SP2 = R3 * S        # 242 flattened conv3 output
PH2 = 288           # padded pairs per core (h-split of 462)
ACT = mybir.ActivationFunctionType
KT_H = H // 128     # 6
ATTM_T = 7          # ceil(792/128)
SPAN_T = 4          # ceil(480/128)


def build_program():
    nc = bacc.Bacc("TRN2", target_bir_lowering=False, debug=False)

    dins = {}

    def din(name, shape, dt=FP32):
        dins[name] = nc.dram_tensor(name, shape, dt, kind="ExternalInput").ap()
        return dins[name]

    f32p = din("f32p", [128, 74], FP32)       # biasp|ident22|g3|brgcnT|btransT
    btb = din("btb", [1, 1024], FP32)         # b_trans|b_rgcn
    bf16p = din("bf16p", [128, 1206], PAIR_DT)  # identp|xg|gspan|gmat|psel|typ|adjf|ones
    wtrans = din("wtrans", [128, KT_H * EMB], SEQ_DT)
    attl = din("attl", [128, SPAN_T * NH * LS], GRAPH_DT)
    xspan = din("xspan", [128, SPAN_T * H], SEQ_DT)
    attm = din("attm", [128, ATTM_T * C], GRAPH_DT)
    xfull = din("xfull", [128, 8 * H], SEQ_DT)
    wrel = din("wrel", [128, 20 * EMB], GRAPH_DT)  # (r,i<4)-tile at col (r*4+i)*512
    wrels = din("wrels", [20, 5 * EMB], GRAPH_DT)  # the 20-row k-tiles, per relation
    w1 = din("w1", [128, 25600], CONV_DT)     # 4 kt-chunks of [128, 25*256]
    w2 = din("w2", [128, 12800], CONV_DT)     # 2 kt-chunks
    w3 = din("w3", [128, 25600], CONV_DT)     # (kt, taphalf) chunks
    shst = din("shst", [E, 2 * PH2], PAIR_DT)
    smp = din("smp", [128, 2 * PH2], PAIR_DT)  # 2 row-tiles of sm
    wht = din("wht", [128, 16 * 1024], PAIR_DT)
    wbil = din("wbil", [128, 8 * 97], PAIR_DT)
    outt = nc.dram_tensor("outt", [97, PH2], FP32, kind="ExternalOutput").ap()

    with tile.TileContext(nc) as tc, ExitStack() as ctx:
        pp = ctx.enter_context(tc.tile_pool(name="persist", bufs=1))
        pst = ctx.enter_context(tc.tile_pool(name="stream", bufs=1))
        pps = ctx.enter_context(tc.tile_pool(name="psum", bufs=8, space="PSUM"))
        pdram = ctx.enter_context(tc.tile_pool(name="dram", bufs=1, space="DRAM"))

        dma = nc.sync.dma_start

        def T(pool, shape, dt, tag, bufs=None):
            return pool.tile(shape, dt, tag=tag, name=tag, bufs=bufs)

        # ---- persistent tiles; two packed small-tensor DMAs + big tensors ----
        bf16p_t = T(pp, [128, 1206], PAIR_DT, "bf16p")
        dma(bf16p_t[:], bf16p)
        f32p_t = T(pp, [128, 74], FP32, "f32p")
        dma(f32p_t[:], f32p)
        btb_t = T(pp, [1, 1024], FP32, "btb")
        dma(btb_t[:], btb)
        identp_t = bf16p_t[:, 0:128]
        xg_t = bf16p_t[:, 128:524]
        gspan_t = bf16p_t[:, 524:644]
        gmat_t = bf16p_t[:, 644:798]
        psel_t = bf16p_t[0:E, 798:815]
        TYP_OFF, ADJ_OFF = 815, 875
        onesb_t = bf16p_t[:, 1205:1206]
        biasp_t = f32p_t[:, 0:22]
        g3_t = f32p_t[0:E * M, 44:66]
        wtrans_t = T(pp, [128, KT_H * EMB], SEQ_DT, "wtrans")
        dma(wtrans_t[:], wtrans)
        attl_t = T(pp, [128, SPAN_T * NH * LS], GRAPH_DT, "attl")
        dma(attl_t[:], attl)
        attm_t = T(pst, [128, ATTM_T * C], GRAPH_DT, "bigreuse", bufs=2)
        dma(attm_t[:, 0:4 * C], attm[:, 0:4 * C])
        xspan_t = T(pp, [128, SPAN_T * H], SEQ_DT, "xspan")
        dma(xspan_t[:], xspan)
        dma(attm_t[:, 4 * C:ATTM_T * C], attm[:, 4 * C:ATTM_T * C])
        xfull_t = T(pst, [128, 8 * H], SEQ_DT, "bigreuse", bufs=2)
        dma(xfull_t[:, 0:4 * H], xfull[:, 0:4 * H])
        dma(xfull_t[:, 4 * H:8 * H], xfull[:, 4 * H:8 * H])
        wrel_t = T(pp, [128, 20 * EMB], GRAPH_DT, "wrel")
        wrels_t = T(pp, [20, 5 * EMB], GRAPH_DT, "wrels")
        dma(wrels_t[:], wrels)
        dma(wrel_t[:, 0:10 * EMB], wrel[:, 0:10 * EMB])
        w1c_t = [T(pst, [128, 6400], CONV_DT, "wconv", bufs=4)
                 for _ in range(4)]
        dma(w1c_t[0][:], w1[:, 0:6400])
        dma(w1c_t[1][:], w1[:, 6400:12800])
        dma(wrel_t[:, 10 * EMB:20 * EMB], wrel[:, 10 * EMB:20 * EMB])

        NODE_GROUPS = [(0, E), (E, E * M), (E + E * M, L)]
        nodes_e = T(pp, [E, 532], GRAPH_DT, "nodes_e")
        nodes_m = T(pp, [E * M, 532], GRAPH_DT, "nodes_m")
        nodes_l = T(pp, [L, 532], GRAPH_DT, "nodes_l")
        node_tiles = [nodes_e, nodes_m, nodes_l]
        for gi, (goff, gsz) in enumerate(NODE_GROUPS):
            nc.vector.tensor_copy(node_tiles[gi][:, 512:532],
                                  bf16p_t[0:gsz, TYP_OFF + gi * 20:TYP_OFF + (gi + 1) * 20])
        adjf_t = [bf16p_t[0:gsz, ADJ_OFF + gi * 110:ADJ_OFF + (gi + 1) * 110]
                  for gi, (goff, gsz) in enumerate(NODE_GROUPS)]

        btrans_bc = T(pp, [128, EMB], FP32, "btrans_bc")
        nc.gpsimd.partition_broadcast(btrans_bc[:], btb_t[0:1, 0:512])

        # preload activation tables off the critical path
        scr_t = T(pp, [1, 4], FP32, "scr")
        nc.vector.memset(scr_t[:], 1.0)
        for fn in (ACT.Exp, ACT.Ln, ACT.Relu, ACT.Tanh):
            nc.scalar.activation(scr_t[:], scr_t[:], fn)

        # chaff psum: keeps the PE pstate at max through sparse phases
        chaff_ps = T(pps, [128, 128], FP32, "ps")

        def chaff(n):
            for _ in range(n):
                nc.tensor.matmul(chaff_ps[:], identp_t, identp_t,
                                 start=True, stop=True)

        chaff(24)

        # conv pad buffers; memset early, off the critical path
        xpad_t = [T(pp, [128, BIN * PW], CONV_DT, f"xpad{mt}") for mt in range(4)]
        pad1_t = [T(pp, [128, B1R * PW], CONV_DT, f"pad1_{mt}") for mt in range(2)]
        pad2_t = [T(pp, [128, B2R * PW], CONV_DT, f"pad2_{mt}") for mt in range(2)]
        for t in xpad_t + pad1_t + pad2_t:
            nc.vector.memset(t[:], 0.0)

        # ---- S2: mention embeddings + entity logsumexp nodes ----
        ps_memb = T(pps, [E * M, EMB], FP32, "ps")
        for kt in range(KT_H):
            nc.tensor.matmul(ps_memb[:], xg_t[:, kt * 66:(kt + 1) * 66],
                             wtrans_t[:, kt * EMB:(kt + 1) * EMB],
                             start=(kt == 0), stop=(kt == KT_H - 1))
        memb_t = T(pp, [E * M, EMB], FP32, "memb")
        nc.vector.tensor_add(memb_t[:], ps_memb[:], btrans_bc[0:E * M, :])
        nc.vector.tensor_copy(nodes_m[:, 0:EMB], memb_t[:])
        ememb_t = T(pp, [E * M, EMB], FP32, "ememb")
        nc.scalar.activation(ememb_t[:], memb_t[:], ACT.Exp)
        ps_ent = T(pps, [E, EMB], FP32, "ps")
        nc.tensor.matmul(ps_ent[:], g3_t, ememb_t[:], start=True, stop=True)
        nc.scalar.activation(nodes_e[:, 0:EMB], ps_ent[:], ACT.Ln)
        chaff(6)

        # ---- S3: link nodes (scale folded into gspan, not xspan) ----
        gsc_t = []
        for i in range(SPAN_T):
            a = T(pst, [128, 1], FP32, "aT", bufs=4)
            nc.vector.tensor_reduce(a[:], attl_t[:, i * 192:(i + 1) * 192],
                                    mybir.AxisListType.X, mybir.AluOpType.add)
            g = T(pp, [128, L], SEQ_DT, f"gsc{i}")
            nc.vector.tensor_scalar_mul(g[:], gspan_t[:, i * L:(i + 1) * L], a[:])
            gsc_t.append(g)
        ps_as = T(pps, [L, 1], FP32, "ps")
        for i in range(SPAN_T):
            nc.tensor.matmul(ps_as[:], gsc_t[i][:], onesb_t,
                             start=(i == 0), stop=(i == SPAN_T - 1))
        asum_t = T(pp, [L, 1], FP32, "asum")
        nc.vector.tensor_scalar_mul(asum_t[:], ps_as[:], 1.0 / (NH * LS))
        ps_lct = [T(pps, [128, L], FP32, "ps") for _ in range(KT_H)]
        for i in range(SPAN_T):
            for mt in range(KT_H):
                nc.tensor.matmul(ps_lct[mt][:],
                                 xspan_t[:, i * H + mt * 128:i * H + (mt + 1) * 128],
                                 gsc_t[i][:],
                                 start=(i == 0), stop=(i == SPAN_T - 1))
        lct_t = []
        for mt in range(KT_H):
            t = T(pp, [128, L], SEQ_DT, f"lct{mt}")
            nc.vector.tensor_scalar_mul(t[:], ps_lct[mt][:], 1.0 / (NH * LS))
            lct_t.append(t)
        bterm_t = T(pp, [L, EMB], FP32, "bterm")
        nc.vector.tensor_scalar_mul(bterm_t[:], btrans_bc[0:L, :], asum_t[:])
        ps_link = T(pps, [L, EMB], FP32, "ps")
        for kt in range(KT_H):
            nc.tensor.matmul(ps_link[:], lct_t[kt][:],
                             wtrans_t[:, kt * EMB:(kt + 1) * EMB],
                             start=(kt == 0), stop=(kt == KT_H - 1))
        nc.vector.tensor_add(nodes_l[:, 0:EMB], ps_link[:], bterm_t[:])
        chaff(6)

        # rsum[e] = sum_rows gmat[row,e] * rowsum(attm[row,:]) — first half
        arsum_t = T(pp, [128, ATTM_T], FP32, "arsum")
        for i in range(4):
            nc.vector.tensor_reduce(arsum_t[:, i:i + 1], attm_t[:, i * C:(i + 1) * C],
                                    mybir.AxisListType.X, mybir.AluOpType.add)
        # adjacency row-normalize (entity destination columns only)
        ps_rsE = T(pps, [1, 5 * E], FP32, "ps")
        for gi, (goff, gsz) in enumerate(NODE_GROUPS):
            nc.tensor.matmul(ps_rsE[:], onesb_t[0:gsz, 0:1], adjf_t[gi],
                             start=(gi == 0), stop=(gi == 2))
        rs_t = T(pp, [1, 5 * E], FP32, "rs")
        nc.vector.tensor_scalar_add(rs_t[:], ps_rsE[:], 1e-5)
        rcp_t = T(pp, [1, 5 * E], FP32, "rcp")
        nc.vector.reciprocal(rcp_t[:], rs_t[:])
        rsbc_t = T(pp, [128, 5 * E], FP32, "rsbc")
        nc.gpsimd.partition_broadcast(rsbc_t[:], rcp_t[:])
        nc.gpsimd.dma_start(w1c_t[2][:], w1[:, 12800:19200])
        rsbcb_t = T(pp, [128, 5 * E], GRAPH_DT, "rsbcb")
        nc.vector.tensor_copy(rsbcb_t[:], rsbc_t[:])
        adjn_t = []
        for gi, (goff, gsz) in enumerate(NODE_GROUPS):
            t = T(pp, [gsz, 5 * E], GRAPH_DT, f"adjn{gi}")
            nc.vector.tensor_mul(t[:], adjf_t[gi], rsbcb_t[0:gsz, :])
            adjn_t.append(t)

        msgE_t = []
        for i in range(5):
            ksz = 128 if i < 4 else 20
            ps = T(pps, [ksz, 5 * E], FP32, "ps")
            for gi, (goff, gsz) in enumerate(NODE_GROUPS):
                nc.tensor.matmul(ps[:], node_tiles[gi][:, i * 128:i * 128 + ksz],
                                 adjn_t[gi][:],
                                 start=(gi == 0), stop=(gi == 2))
            t = T(pp, [ksz, 5 * E], GRAPH_DT, f"msgE{i}")
            nc.vector.tensor_copy(t[:], ps[:])
            msgE_t.append(t)

        # ---- S4: eaT directly transposed; e_ctx = (ean@X)@W + rn*b ----
        for i in range(4, ATTM_T):
            nc.vector.tensor_reduce(arsum_t[:, i:i + 1], attm_t[:, i * C:(i + 1) * C],
                                    mybir.AxisListType.X, mybir.AluOpType.add)
        arsumb_t = T(pp, [128, ATTM_T], GRAPH_DT, "arsumb")
        nc.vector.tensor_copy(arsumb_t[:], arsum_t[:])
        ps_ct = [T(pps, [128, E], FP32, "ps") for _ in range(8)]
        for i in range(4):
            for ct in range(8):
                nc.tensor.matmul(ps_ct[ct][:],
                                 attm_t[:, i * C + ct * 128:i * C + (ct + 1) * 128],
                                 gmat_t[:, i * E:(i + 1) * E],
                                 start=(i == 0), stop=(i == 3))
        eaP_t = []
        for ct in range(8):
            t = T(pst, [128, E], FP32, "eaP", bufs=8)
            nc.vector.tensor_copy(t[:], ps_ct[ct][:])
            eaP_t.append(t)
        chaff(48)
        ps_ct2 = [T(pps, [128, E], FP32, "ps") for _ in range(8)]
        for i in range(4, ATTM_T):
            for ct in range(8):
                nc.tensor.matmul(ps_ct2[ct][:],
                                 attm_t[:, i * C + ct * 128:i * C + (ct + 1) * 128],
                                 gmat_t[:, i * E:(i + 1) * E],
                                 start=(i == 4), stop=(i == ATTM_T - 1))
        eaTb_t = []
        for ct in range(8):
            t = T(pp, [128, E], GRAPH_DT, f"eaTb{ct}")
            nc.vector.tensor_add(t[:], ps_ct2[ct][:], eaP_t[ct][:])
            eaTb_t.append(t)
        ps_rs = T(pps, [E, 1], FP32, "ps")
        for i in range(ATTM_T):
            nc.tensor.matmul(ps_rs[:], gmat_t[:, i * E:(i + 1) * E],
                             arsumb_t[:, i:i + 1],
                             start=(i == 0), stop=(i == ATTM_T - 1))
        rsum_t = T(pp, [E, 1], FP32, "rsum")
        nc.vector.tensor_scalar_add(rsum_t[:], ps_rs[:], 1e-5)
        recip_t = T(pp, [E, 1], FP32, "recip")
        nc.vector.reciprocal(recip_t[:], rsum_t[:])
        # rn = rsum/(rsum+eps) = 1 - eps*recip
        # recip as a row vector broadcast across partitions
        ps_rt = T(pps, [1, E], FP32, "ps")
        nc.tensor.transpose(ps_rt[:], recip_t[:], f32p_t[0:E, 22:44])
        recipT_t = T(pp, [1, E], FP32, "recipT")
        nc.vector.tensor_copy(recipT_t[:], ps_rt[:])
        recipT_bc = T(pp, [128, E], FP32, "recipT_bc")
        nc.gpsimd.partition_broadcast(recipT_bc[:], recipT_t[:])

        # eanXT[h,e] tiles directly (no transposes), normalized per entity
        ps_xT = [T(pps, [128, E], FP32, "ps") for _ in range(KT_H)]
        for ct in range(8):
            for ht in range(KT_H):
                nc.tensor.matmul(ps_xT[ht][:],
                                 xfull_t[:, ct * H + ht * 128:ct * H + (ht + 1) * 128],
                                 eaTb_t[ct][:], start=(ct == 0), stop=(ct == 7))
        eanXT_t = []
        for ht in range(KT_H):
            t = T(pp, [128, E], SEQ_DT, f"eanXT{ht}")
            nc.vector.tensor_mul(t[:], ps_xT[ht][:], recipT_bc[:])
            eanXT_t.append(t)
        chaff(6)
        ps_ectxT = [T(pps, [128, E], FP32, "ps") for _ in range(4)]
        for ht in range(KT_H):
            for mt in range(4):
                nc.tensor.matmul(ps_ectxT[mt][:],
                                 wtrans_t[:, ht * EMB + mt * 128:ht * EMB + (mt + 1) * 128],
                                 eanXT_t[ht][:], start=(ht == 0), stop=(ht == KT_H - 1))
        # rnT = rowsum(ean) as a row = 1 - eps*recipT
        rnT_t = T(pp, [1, E], FP32, "rnT")
        nc.vector.tensor_scalar_mul(rnT_t[:], recipT_t[:], -1e-5)
        nc.vector.tensor_scalar_add(rnT_t[:], rnT_t[:], 1.0)
        rnT_bc = T(pp, [128, E], FP32, "rnT_bc")
        nc.gpsimd.partition_broadcast(rnT_bc[:], rnT_t[:])
        ectxT_t = []
        for mt in range(4):
            bt_ = T(pst, [128, E], FP32, "ebias", bufs=2)
            nc.vector.tensor_scalar_mul(bt_[:], rnT_bc[:], f32p_t[:, 70 + mt:71 + mt])
            t = T(pp, [128, E], PAIR_DT, f"ectxT{mt}")
            nc.vector.tensor_add(t[:], ps_ectxT[mt][:], bt_[:])
            ectxT_t.append(t)
        ectxb_t = T(pp, [E, EMB], PAIR_DT, "ectxb")
        for mt in range(4):
            ps = T(pps, [E, 64], FP32, "ps")
            psb = ps[:].bitcast(PAIR_DT)
            nc.tensor.transpose(psb, ectxT_t[mt][:], bf16p_t[:, 0:128])
            nc.vector.tensor_copy(ectxb_t[:, mt * 128:(mt + 1) * 128], psb)

        # ---- S5: RGCN (entity rows only) ----
        ps_gcnT = [T(pps, [128, E], FP32, "ps") for _ in range(4)]
        term = 0
        for r in range(5):
            for i in range(4):
                for mt in range(4):
                    nc.tensor.matmul(
                        ps_gcnT[mt][:],
                        wrel_t[:, (r * 4 + i) * EMB + mt * 128:(r * 4 + i) * EMB + (mt + 1) * 128],
                        msgE_t[i][:, r * E:(r + 1) * E],
                        start=(term == 0), stop=False)
                term += 1
        for r in range(5):
            for mt in range(4):
                nc.tensor.matmul(ps_gcnT[mt][:],
                                 wrels_t[0:20, r * EMB + mt * 128:r * EMB + (mt + 1) * 128],
                                 msgE_t[4][0:20, r * E:(r + 1) * E],
                                 start=False, stop=(r == 4))
        nc.gpsimd.dma_start(w1c_t[3][:], w1[:, 19200:25600])
        gcnT_t = []
        for mt in range(4):
            t = T(pp, [128, E], PAIR_DT, f"gcnT{mt}")
            nc.scalar.activation(t[:], ps_gcnT[mt][:], ACT.Relu,
                                 bias=f32p_t[:, 66 + mt:67 + mt])
            gcnT_t.append(t)
        entb_t = T(pp, [E, EMB], PAIR_DT, "entb")
        for mt in range(4):
            ps = T(pps, [E, 64], FP32, "ps")
            psb = ps[:].bitcast(PAIR_DT)
            nc.tensor.transpose(psb, gcnT_t[mt][:], bf16p_t[:, 0:128])
            nc.vector.tensor_copy(entb_t[:, mt * 128:(mt + 1) * 128], psb)

        # ---- S6: relation map x (bf16 transposes + row selection) ----
        ps_c1 = [T(pps, [128, R1 * S], FP32, "ps") for _ in range(2)]
        entT_t, ectxTv_t = gcnT_t, ectxT_t
        entS_t, ectxS_t = [], []
        for mt in range(4):
            for src_t, dst_list, nf in ((entb_t, entS_t, "entS"),
                                        (ectxb_t, ectxS_t, "ectxS")):
                ps = T(pps, [128, 17], FP32, "ps")
                nc.tensor.matmul(ps[:], src_t[:, mt * 128:(mt + 1) * 128], psel_t,
                                 start=True, stop=True)
                t = T(pp, [128, 17], PAIR_DT, f"{nf}{mt}")
                nc.vector.tensor_copy(t[:], ps[:])
                dst_list.append(t)
            t1 = T(pst, [128, 17 * S], FP32, "xtmp", bufs=2)
            nc.vector.tensor_mul(
                t1[:].rearrange("p (a b) -> p a b", a=17, b=S),
                entS_t[mt][:].unsqueeze(2).to_broadcast((128, 17, S)),
                entT_t[mt][:].unsqueeze(1).to_broadcast((128, 17, S)))
            t2 = T(pst, [128, 17 * S], FP32, "xtmp", bufs=2)
            nc.vector.tensor_mul(
                t2[:].rearrange("p (a b) -> p a b", a=17, b=S),
                ectxS_t[mt][:].unsqueeze(2).to_broadcast((128, 17, S)),
                ectxTv_t[mt][:].unsqueeze(1).to_broadcast((128, 17, S)))
            inner = xpad_t[mt][:].rearrange("p (a b) -> p a b", a=BIN, b=PW)[
                :, 2:2 + 17, 2:2 + S]
            nc.vector.tensor_add(inner,
                                 t1[:].rearrange("p (a b) -> p a b", a=17, b=S),
                                 t2[:].rearrange("p (a b) -> p a b", a=17, b=S))
            if mt == 0:
                chaff(16)

        # conv1: 512 -> 256
        for kt in range(4):
            w = w1c_t[kt]
            for tap in range(25):
                di, dj = divmod(tap, 5)
                rhs = xpad_t[kt][:].rearrange("p (a b) -> p a b", a=BIN, b=PW)[
                    :, di:di + R1, dj:dj + S]
                for mt in range(2):
                    nc.tensor.matmul(ps_c1[mt][:],
                                     w[:, tap * 256 + mt * 128:tap * 256 + (mt + 1) * 128],
                                     rhs, start=(kt == 0 and tap == 0),
                                     stop=(kt == 3 and tap == 24))
        for mt in range(2):
            inner = pad1_t[mt][:].rearrange("p (a b) -> p a b", a=B1R, b=PW)[
                :, 2:2 + R1, 2:2 + S]
            nc.scalar.activation(inner,
                                 ps_c1[mt][:].rearrange("p (a b) -> p a b", a=R1, b=S),
                                 ACT.Relu, bias=biasp_t[:, mt:mt + 1])

        # conv2: 256 -> 256
        ps_c2 = [T(pps, [128, R2 * S], FP32, "ps") for _ in range(2)]
        for kt in range(2):
            w = T(pst, [128, 6400], CONV_DT, "wconv", bufs=4)
            dma(w[:], w2[:, kt * 6400:(kt + 1) * 6400])
            for tap in range(25):
                di, dj = divmod(tap, 5)
                rhs = pad1_t[kt][:].rearrange("p (a b) -> p a b", a=B1R, b=PW)[
                    :, di:di + R2, dj:dj + S]
                for mt in range(2):
                    nc.tensor.matmul(ps_c2[mt][:],
                                     w[:, tap * 256 + mt * 128:tap * 256 + (mt + 1) * 128],
                                     rhs, start=(kt == 0 and tap == 0),
                                     stop=(kt == 1 and tap == 24))
        for mt in range(2):
            inner = pad2_t[mt][:].rearrange("p (a b) -> p a b", a=B2R, b=PW)[
                :, 2:2 + R2, 2:2 + S]
            nc.scalar.activation(inner,
                                 ps_c2[mt][:].rearrange("p (a b) -> p a b", a=R2, b=S),
                                 ACT.Relu, bias=biasp_t[:, 2 + mt:3 + mt])

        # wht streamed now so its 4.2MB overlaps conv2/conv3 compute
        whtc = []
        for c in range(2):
            t = T(pst, [128, 8 * 1024], PAIR_DT, "bigreuse", bufs=2)
            dma(t[:], wht[:, c * 8192:(c + 1) * 8192])
            whtc.append(t)

        # retire chaff psum so it isn't dead code
        warm_sb = T(pp, [128, 128], FP32, "warm_sb")
        nc.vector.tensor_copy(warm_sb[:], chaff_ps[:])
        warm_dram = pdram.tile([128, 128], FP32, name="warm_dram")
        dma(warm_dram[:], warm_sb[:])

        # conv3: 256 -> 512, four (kt, taphalf) chunks
        x3_t = [T(pp, [128, SP2], PAIR_DT, f"x3_{mt}") for mt in range(4)]
        ps_c3 = [T(pps, [128, SP2], FP32, "ps") for _ in range(4)]
        for kt in range(2):
            for taps in (range(0, 13), range(13, 25)):
                w = T(pst, [128, len(taps) * EMB], CONV_DT, "wconv", bufs=4)
                dma(w[:], w3[:, (kt * 25 + taps.start) * EMB:(kt * 25 + taps.stop) * EMB])
                for tj, tap in enumerate(taps):
                    di, dj = divmod(tap, 5)
                    rhs = pad2_t[kt][:].rearrange("p (a b) -> p a b", a=B2R, b=PW)[
                        :, di:di + R3, dj:dj + S]
                    for mt in range(4):
                        nc.tensor.matmul(ps_c3[mt][:],
                                         w[:, tj * EMB + mt * 128:tj * EMB + (mt + 1) * 128],
                                         rhs, start=(kt == 0 and tap == 0),
                                         stop=(kt == 1 and tap == 24))
        for mt in range(4):
            nc.scalar.activation(x3_t[mt][:], ps_c3[mt][:], ACT.Relu,
                                 bias=biasp_t[:, 4 + mt:5 + mt])

        # ---- S7: pair features + classifier ----
        SP_TILES = [(0, 128), (128, SP2 - 128)]
        x3T_t = [T(pp, [sz, EMB], PAIR_DT, f"x3T{i}")
                 for i, (off, sz) in enumerate(SP_TILES)]
        for i, (off, sz) in enumerate(SP_TILES):
            for src in range(4):
                ps = T(pps, [sz, 64], FP32, "ps")
                psb = ps[:].bitcast(PAIR_DT)
                nc.tensor.transpose(psb, x3_t[src][:, off:off + sz], bf16p_t[:, 0:128])
                nc.vector.tensor_copy(x3T_t[i][:, src * 128:(src + 1) * 128], psb)
        chaff(20)

        shst_t = T(pp, [E, 2 * PH2], PAIR_DT, "shst")
        dma(shst_t[:], shst)
        smp_t = T(pp, [128, 2 * PH2], PAIR_DT, "smp")
        dma(smp_t[:], smp)

        featT = [None] * 16
        for src in range(2):
            for mt in range(4):
                ps = T(pps, [128, PH2], FP32, "ps")
                nc.tensor.matmul(ps[:], entb_t[:, mt * 128:(mt + 1) * 128],
                                 shst_t[:, src * PH2:(src + 1) * PH2],
                                 start=True, stop=True)
                t = T(pp, [128, PH2], PAIR_DT, f"featT{4 * src + mt}")
                nc.vector.tensor_copy(t[:], ps[:])
                featT[4 * src + mt] = t
        for mt in range(4):
            ps = T(pps, [128, PH2], FP32, "ps")
            for i, (off, sz) in enumerate(SP_TILES):
                nc.tensor.matmul(ps[:], x3T_t[i][0:sz, mt * 128:(mt + 1) * 128],
                                 smp_t[0:sz, i * PH2:(i + 1) * PH2],
                                 start=(i == 0), stop=(i == 1))
            t = T(pp, [128, PH2], PAIR_DT, f"featT{8 + mt}")
            nc.vector.tensor_copy(t[:], ps[:])
            featT[8 + mt] = t
        for mt in range(4):
            t = T(pp, [128, PH2], PAIR_DT, f"featT{12 + mt}")
            nc.vector.tensor_mul(t[:], featT[mt][:], featT[4 + mt][:])
            featT[12 + mt] = t

        chaff(12)
        ps_ht = [T(pps, [128, PH2], FP32, "ps") for _ in range(8)]
        for kt in range(16):
            wv = whtc[kt // 8]
            for mt in range(8):
                nc.tensor.matmul(
                    ps_ht[mt][:],
                    wv[:, (kt % 8) * 1024 + mt * 128:(kt % 8) * 1024 + (mt + 1) * 128],
                    featT[kt][:], start=(kt == 0), stop=(kt == 15))
        htT_t = []
        for mt in range(8):
            t = T(pp, [128, PH2], PAIR_DT, f"htT{mt}")
            nc.scalar.activation(t[:], ps_ht[mt][:], ACT.Tanh,
                                 bias=biasp_t[:, 12 + mt:13 + mt])
            htT_t.append(t)

        chaff(6)
        wbil_t = T(pp, [128, 8 * 97], PAIR_DT, "wbil")
        dma(wbil_t[:], wbil)
        ps_out = T(pps, [97, PH2], FP32, "ps")
        for kt in range(8):
            nc.tensor.matmul(ps_out[:], wbil_t[:, kt * 97:(kt + 1) * 97],
                             htT_t[kt][:], start=(kt == 0), stop=(kt == 7))
        out_t = T(pp, [97, PH2], FP32, "out")
        nc.vector.tensor_scalar_add(out_t[:], ps_out[:], biasp_t[0:97, 20:21])
        dma(outt, out_t[:])

    nc.compile()
    return nc


_PROG = None


def _get_prog():
    global _PROG
    if _PROG is None:
        _PROG = build_program()
    return _PROG


def _np(dt):
    return _NPDT[dt]


def _pack_rows(a, ntiles, dt):
    """[ntiles*128, W] (zero-padded) -> [128, ntiles*W] with tile i at
    column block i."""
    r, w = a.shape
    pad = ntiles * 128 - r
    if pad:
        a = np.concatenate([a, np.zeros((pad, w), a.dtype)], axis=0)
    return np.ascontiguousarray(
        a.reshape(ntiles, 128, w).transpose(1, 0, 2).reshape(128, ntiles * w),
        _np(dt))


def _pack_conv(w, flip, dt):
    """conv weight OIHW -> tap-major per-kt chunks [128, ...]."""
    w = np.asarray(w, np.float32)
    if flip:
        w = w[:, :, ::-1, :]
    oc, ic, _, _ = w.shape
    t = w.transpose(2, 3, 1, 0).reshape(25, ic, oc)   # (tap, ic, oc)
    nkt = ic // 128
    chunks = [np.ascontiguousarray(
        t[:, kt * 128:(kt + 1) * 128, :].transpose(1, 0, 2).reshape(128, 25 * oc))
        for kt in range(nkt)]
    return np.ascontiguousarray(np.concatenate(chunks, axis=1), _np(dt))


def _pack_conv3(w, flip, dt):
    """conv3 weights as (kt, taphalf) chunks: [128, 25*512] per kt with
    taps in order — column block (kt*25 + tap)*512."""
    w = np.asarray(w, np.float32)
    if flip:
        w = w[:, :, ::-1, :]
    t = w.transpose(2, 3, 1, 0).reshape(25, IC, EMB)
    chunks = [np.ascontiguousarray(
        t[:, kt * 128:(kt + 1) * 128, :].transpose(1, 0, 2).reshape(128, 25 * EMB))
        for kt in range(2)]
    return np.ascontiguousarray(np.concatenate(chunks, axis=1), _np(dt))


def _shared_inputs(inputs):
    f32 = np.float32
    sh = {}
    fp = np.zeros((128, 74), f32)
    fp[:, 0] = np.asarray(inputs["conv1_b"], f32)[0:128]
    fp[:, 1] = np.asarray(inputs["conv1_b"], f32)[128:256]
    fp[:, 2] = np.asarray(inputs["conv2_b"], f32)[0:128]
    fp[:, 3] = np.asarray(inputs["conv2_b"], f32)[128:256]
    for mt in range(4):
        fp[:, 4 + mt] = np.asarray(inputs["conv3_b"], f32)[mt * 128:(mt + 1) * 128]
    for mt in range(8):
        fp[:, 12 + mt] = np.asarray(inputs["ht_b"], f32)[mt * 128:(mt + 1) * 128]
    fp[0:97, 20] = np.asarray(inputs["bil_b"], f32)
    fp[:, 21] = 1.0
    fp[0:22, 22:44] = np.eye(22, dtype=f32)
    fp[0:E * M, 44:66] = np.kron(np.eye(E, dtype=f32), np.ones((M, 1), f32))
    for mt in range(4):
        fp[:, 66 + mt] = np.asarray(inputs["b_rgcn"], f32)[mt * 128:(mt + 1) * 128]
        fp[:, 70 + mt] = np.asarray(inputs["b_trans"], f32)[mt * 128:(mt + 1) * 128]
    sh["f32p"] = fp
    bb = np.zeros((1, 1024), f32)
    bb[0, 0:512] = np.asarray(inputs["b_trans"], f32)
    bb[0, 512:1024] = np.asarray(inputs["b_rgcn"], f32)
    sh["btb"] = bb
    bt = np.zeros((128, 1206), np.float32)
    bt[:, 0:128] = np.eye(128, dtype=f32)
    bt[:, 524:644] = _pack_rows(np.kron(np.eye(L, dtype=f32),
                                        np.ones((LS, 1), f32)), SPAN_T, FP32)
    bt[:, 644:798] = _pack_rows(np.kron(np.eye(E, dtype=f32),
                                        np.ones((M * NH, 1), f32) / (M * NH)),
                                ATTM_T, FP32)
    bt[:, 1205] = 1.0
    sh["bf16p_base"] = bt
    sh["wtrans"] = _pack_rows(np.asarray(inputs["W_trans"], f32), KT_H, SEQ_DT)
    # wrel: 5 relations (4 + self); big k-tiles [128] and the 20-row tail
    wr = np.zeros((5, 532, EMB), f32)
    wr[0:4] = np.asarray(inputs["W_rel"], f32)
    wr[4] = np.asarray(inputs["W_self"], f32)
    wrb = wr[:, 0:512, :].reshape(5, 4, 128, EMB)   # (r, i, 128, 512)
    sh["wrel"] = np.ascontiguousarray(
        wrb.transpose(2, 0, 1, 3).reshape(128, 20 * EMB), _np(GRAPH_DT))
    sh["wrels"] = np.ascontiguousarray(
        wr[:, 512:532, :].transpose(1, 0, 2).reshape(20, 5 * EMB), _np(GRAPH_DT))
    sh["w1"] = [_pack_conv(inputs["conv1_w"], fl, CONV_DT) for fl in (0, 1)]
    sh["w2"] = [_pack_conv(inputs["conv2_w"], fl, CONV_DT) for fl in (0, 1)]
    sh["w3"] = [_pack_conv3(inputs["conv3_w"], fl, CONV_DT) for fl in (0, 1)]
    sh["wht"] = _pack_rows(np.asarray(inputs["ht_W"], f32), 16, PAIR_DT)
    sh["wbil"] = _pack_rows(np.asarray(inputs["bil_W"], f32), 8, PAIR_DT)
    psel = []
    for fl in (0, 1):
        pm = np.zeros((E, 17), f32)
        for r in range(17):
            pm[(21 - r) if fl else r, r] = 1.0
        psel.append(pm)
    sh["psel"] = psel
    return sh


def _pair_idx(hts_b, hh):
    h = np.asarray(hts_b)[:, 0]
    mask = (h <= 10) if hh == 0 else (h >= 11)
    idx = np.nonzero(mask)[0]
    if len(idx) > PH2:
        raise RuntimeError(f"pair overflow: {len(idx)} > {PH2}")
    return idx


def _core_inputs(inputs, shared, b, hh):
    f32 = np.float32
    X = np.asarray(inputs["sequence_output"][b], f32)
    att = np.asarray(inputs["attention"][b], f32)
    adj = np.asarray(inputs["adjacency"][b], f32)
    mf = np.asarray(inputs["mention_idx"][b]).reshape(-1).astype(np.int64)
    ls = np.asarray(inputs["link_start"][b]).reshape(-1).astype(np.int64)
    ntypes = np.asarray(inputs["node_types"][b]).astype(np.int64)
    hts = np.asarray(inputs["hts"][b]).astype(np.int64)

    m = {k: shared[k] for k in
         ("f32p", "btb", "wtrans", "wrel", "wrels", "wht", "wbil")}
    m["w1"] = shared["w1"][hh]
    m["w2"] = shared["w2"][hh]
    m["w3"] = shared["w3"][hh]
    bt = shared["bf16p_base"].copy()
    bt[:, 128:524] = _pack_rows(np.ascontiguousarray(X[mf].T), KT_H, FP32)
    bt[0:E, 798:815] = shared["psel"][hh]
    te = np.asarray(inputs["type_embed"], f32)[ntypes]
    adjc = np.concatenate([adj[r].T[:, 0:E] for r in range(4)]
                          + [np.eye(NN, E, dtype=f32)], axis=1)
    for gi, (goff, gsz) in enumerate([(0, E), (E, E * M), (E + E * M, L)]):
        bt[0:gsz, 815 + gi * 20:835 + gi * 20] = te[goff:goff + gsz]
        bt[0:gsz, 875 + gi * 110:985 + gi * 110] = adjc[goff:goff + gsz]
    m["bf16p"] = np.ascontiguousarray(bt, _np(PAIR_DT))
    pos = ls[:, None] + np.arange(LS)
    m["xspan"] = _pack_rows(X[pos.reshape(-1)], SPAN_T, SEQ_DT)
    m["xfull"] = _pack_rows(X, 8, SEQ_DT)
    rows = att[:, mf, :]
    m["attm"] = _pack_rows(rows.transpose(1, 0, 2).reshape(E * M * NH, C),
                           ATTM_T, GRAPH_DT)
    attl = np.empty((L * LS, NH * LS), f32)
    for l in range(L):
        blk = att[:, pos[l], :][:, :, pos[l]]
        attl[l * LS:(l + 1) * LS, :] = blk.transpose(2, 0, 1).reshape(LS, NH * LS)
    m["attl"] = _pack_rows(attl, SPAN_T, GRAPH_DT)

    idx = _pair_idx(hts, hh)
    pr = hts[idx]
    n = len(idx)
    shm = np.zeros((E, 2 * PH2), f32)
    shm[pr[:, 0], np.arange(n)] = 1.0
    shm[pr[:, 1], PH2 + np.arange(n)] = 1.0
    m["shst"] = np.ascontiguousarray(shm, _np(PAIR_DT))
    loc_r = pr[:, 0] if hh == 0 else (21 - pr[:, 0])
    smm = np.zeros((SP2, PH2), f32)
    smm[loc_r * S + pr[:, 1], np.arange(n)] = 1.0
    m["smp"] = _pack_rows(smm, 2, PAIR_DT)
    return m


def kernel(**inputs):
    nc = _get_prog()
    shared = _shared_inputs(inputs)
    in_maps = []
    for b in range(B):
        for hh in range(2):
            in_maps.append(_core_inputs(inputs, shared, b, hh))
    res = run_bass_kernel_spmd(nc, in_maps, core_ids=list(range(8)))
    out = np.empty((B, P, 97), np.float32)
    for b in range(B):
        for hh in range(2):
            idx = _pair_idx(np.asarray(inputs["hts"][b]), hh)
            r = np.asarray(res.results[2 * b + hh]["outt"], np.float32)
            out[b, idx, :] = r[:, 0:len(idx)].T
    return out


# revision 32
# speedup vs baseline: 1.0113x; 1.0113x over previous
"""Trainium2 Bass kernel for nn_DocREModel (8-core SPMD).

Sharding: data-parallel over the 4 documents x 2 halves = 8 cores.
Each doc's two cores duplicate the (cheap) graph phase, then split the
expensive conv reasoning stack SPATIALLY: core A computes output rows
0..10 of the 22x22 relation map, core B rows 11..21.  B works on a
row-FLIPPED frame with row-flipped conv taps so both cores run the
identical instruction stream (SPMD) -- only the data differs.  Pairs
are split by head entity (h<=10 vs h>=11), padded to PH2=288 columns.

e_ctx is reassociated:  ean @ (X @ W + b) == (ean @ X) @ W + rowsum(ean)*b,
which removes the full [1024,512] seq transform.

All floating-point arithmetic runs on device.  Host does only
index-driven data movement: batch slicing, transposes, row gathers at
integer indices, one-hot/selector construction, and layout packing.

DMA discipline: every sizable tensor is packed host-side as [128, W]
with per-partition-contiguous rows so each tensor is ONE dma_start
(the SP sequencer pays ~565ns per dma_start; the baseline's 334 DMAs
serialized 205us of sync time).
"""

import numpy as np
from contextlib import ExitStack

import concourse.bass as bass
import concourse.bacc as bacc
import concourse.tile as tile
import concourse.mybir as mybir
from concourse.bass_utils import run_bass_kernel_spmd

import ml_dtypes

FP32 = mybir.dt.float32
BF16 = mybir.dt.bfloat16

SEQ_DT = BF16
CONV_DT = BF16
PAIR_DT = BF16
GRAPH_DT = BF16

_NPDT = {FP32: np.float32, BF16: ml_dtypes.bfloat16}

B, C, H, NH = 4, 1024, 768, 12
E, M, L, LS = 22, 3, 30, 16
NN, EMB = 118, 512
P = 462
IC = 256
S = 22              # spatial side of relation map
PW = 26             # padded column width (2+22+2)
R1, R2, R3 = 15, 13, 11   # conv1/2/3 computed rows (half + halo)
BIN, B1R, B2R = 21, 19, 17  # padded row counts of conv input/1/2 buffers
SP2 = R3 * S        # 242 flattened conv3 output
PH2 = 288           # padded pairs per core (h-split of 462)
ACT = mybir.ActivationFunctionType
KT_H = H // 128     # 6
ATTM_T = 7          # ceil(792/128)
SPAN_T = 4          # ceil(480/128)


def build_program():
    nc = bacc.Bacc("TRN2", target_bir_lowering=False, debug=False)

    dins = {}

    def din(name, shape, dt=FP32):
        dins[name] = nc.dram_tensor(name, shape, dt, kind="ExternalInput").ap()
        return dins[name]

    f32p = din("f32p", [128, 74], FP32)       # biasp|ident22|g3|brgcnT|btransT
    btb = din("btb", [1, 1024], FP32)         # b_trans|b_rgcn
    bf16p = din("bf16p", [128, 1206], PAIR_DT)  # identp|xg|gspan|gmat|psel|typ|adjf|ones
    wtrans = din("wtrans", [128, KT_H * EMB], SEQ_DT)
    attl = din("attl", [128, SPAN_T * NH * LS], GRAPH_DT)
    xspan = din("xspan", [128, SPAN_T * H], SEQ_DT)
    attm = din("attm", [128, ATTM_T * C], GRAPH_DT)
    xfull = din("xfull", [128, 8 * H], SEQ_DT)
    wrel = din("wrel", [128, 20 * EMB], GRAPH_DT)  # (r,i<4)-tile at col (r*4+i)*512
    wrels = din("wrels", [20, 5 * EMB], GRAPH_DT)  # the 20-row k-tiles, per relation
    w1 = din("w1", [128, 25600], CONV_DT)     # 4 kt-chunks of [128, 25*256]
    w2 = din("w2", [128, 12800], CONV_DT)     # 2 kt-chunks
    w3 = din("w3", [128, 25600], CONV_DT)     # (kt, taphalf) chunks
    shst = din("shst", [E, 2 * PH2], PAIR_DT)
    smp = din("smp", [128, 2 * PH2], PAIR_DT)  # 2 row-tiles of sm
    wht = din("wht", [128, 16 * 1024], PAIR_DT)
    wbil = din("wbil", [128, 8 * 97], PAIR_DT)
    outt = nc.dram_tensor("outt", [97, PH2], FP32, kind="ExternalOutput").ap()

    with tile.TileContext(nc) as tc, ExitStack() as ctx:
        pp = ctx.enter_context(tc.tile_pool(name="persist", bufs=1))
        pst = ctx.enter_context(tc.tile_pool(name="stream", bufs=1))
        pps = ctx.enter_context(tc.tile_pool(name="psum", bufs=8, space="PSUM"))
        pdram = ctx.enter_context(tc.tile_pool(name="dram", bufs=1, space="DRAM"))

        dma = nc.sync.dma_start

        def T(pool, shape, dt, tag, bufs=None):
            return pool.tile(shape, dt, tag=tag, name=tag, bufs=bufs)

        # ---- persistent tiles; two packed small-tensor DMAs + big tensors ----
        bf16p_t = T(pp, [128, 1206], PAIR_DT, "bf16p")
        dma(bf16p_t[:], bf16p)
        f32p_t = T(pp, [128, 74], FP32, "f32p")
        dma(f32p_t[:], f32p)
        btb_t = T(pp, [1, 1024], FP32, "btb")
        dma(btb_t[:], btb)
        identp_t = bf16p_t[:, 0:128]
        xg_t = bf16p_t[:, 128:524]
        gspan_t = bf16p_t[:, 524:644]
        gmat_t = bf16p_t[:, 644:798]
        psel_t = bf16p_t[0:E, 798:815]
        TYP_OFF, ADJ_OFF = 815, 875
        onesb_t = bf16p_t[:, 1205:1206]
        biasp_t = f32p_t[:, 0:22]
        g3_t = f32p_t[0:E * M, 44:66]
        wtrans_t = T(pp, [128, KT_H * EMB], SEQ_DT, "wtrans")
        dma(wtrans_t[:], wtrans)
        attl_t = T(pp, [128, SPAN_T * NH * LS], GRAPH_DT, "attl")
        dma(attl_t[:], attl)
        attm_t = T(pst, [128, ATTM_T * C], GRAPH_DT, "bigreuse", bufs=2)
        dma(attm_t[:, 0:4 * C], attm[:, 0:4 * C])
        xspan_t = T(pp, [128, SPAN_T * H], SEQ_DT, "xspan")
        dma(xspan_t[:], xspan)
        dma(attm_t[:, 4 * C:ATTM_T * C], attm[:, 4 * C:ATTM_T * C])
        xfull_t = T(pst, [128, 8 * H], SEQ_DT, "bigreuse", bufs=2)
        dma(xfull_t[:, 0:4 * H], xfull[:, 0:4 * H])
        dma(xfull_t[:, 4 * H:8 * H], xfull[:, 4 * H:8 * H])
        wrel_t = T(pp, [128, 20 * EMB], GRAPH_DT, "wrel")
        wrels_t = T(pp, [20, 5 * EMB], GRAPH_DT, "wrels")
        dma(wrels_t[:], wrels)
        dma(wrel_t[:, 0:10 * EMB], wrel[:, 0:10 * EMB])
        w1c_t = [T(pst, [128, 6400], CONV_DT, "wconv", bufs=4)
                 for _ in range(4)]
        dma(w1c_t[0][:], w1[:, 0:6400])
        dma(wrel_t[:, 10 * EMB:20 * EMB], wrel[:, 10 * EMB:20 * EMB])
        dma(w1c_t[1][:], w1[:, 6400:12800])

        NODE_GROUPS = [(0, E), (E, E * M), (E + E * M, L)]
        nodes_e = T(pp, [E, 532], GRAPH_DT, "nodes_e")
        nodes_m = T(pp, [E * M, 532], GRAPH_DT, "nodes_m")
        nodes_l = T(pp, [L, 532], GRAPH_DT, "nodes_l")
        node_tiles = [nodes_e, nodes_m, nodes_l]
        for gi, (goff, gsz) in enumerate(NODE_GROUPS):
            nc.vector.tensor_copy(node_tiles[gi][:, 512:532],
                                  bf16p_t[0:gsz, TYP_OFF + gi * 20:TYP_OFF + (gi + 1) * 20])
        adjf_t = [bf16p_t[0:gsz, ADJ_OFF + gi * 110:ADJ_OFF + (gi + 1) * 110]
                  for gi, (goff, gsz) in enumerate(NODE_GROUPS)]

        btrans_bc = T(pp, [128, EMB], FP32, "btrans_bc")
        nc.gpsimd.partition_broadcast(btrans_bc[:], btb_t[0:1, 0:512])

        # preload activation tables off the critical path
        scr_t = T(pp, [1, 4], FP32, "scr")
        nc.vector.memset(scr_t[:], 1.0)
        for fn in (ACT.Exp, ACT.Ln, ACT.Relu, ACT.Tanh):
            nc.scalar.activation(scr_t[:], scr_t[:], fn)

        # chaff psum: keeps the PE pstate at max through sparse phases
        chaff_ps = T(pps, [128, 128], FP32, "ps")

        def chaff(n):
            for _ in range(n):
                nc.tensor.matmul(chaff_ps[:], identp_t, identp_t,
                                 start=True, stop=True)

        chaff(24)

        # conv pad buffers; memset early, off the critical path
        xpad_t = [T(pp, [128, BIN * PW], CONV_DT, f"xpad{mt}") for mt in range(4)]
        pad1_t = [T(pp, [128, B1R * PW], CONV_DT, f"pad1_{mt}") for mt in range(2)]
        pad2_t = [T(pp, [128, B2R * PW], CONV_DT, f"pad2_{mt}") for mt in range(2)]
        for t in xpad_t + pad1_t + pad2_t:
            nc.vector.memset(t[:], 0.0)

        # ---- S2: mention embeddings + entity logsumexp nodes ----
        ps_memb = T(pps, [E * M, EMB], FP32, "ps")
        for kt in range(KT_H):
            nc.tensor.matmul(ps_memb[:], xg_t[:, kt * 66:(kt + 1) * 66],
                             wtrans_t[:, kt * EMB:(kt + 1) * EMB],
                             start=(kt == 0), stop=(kt == KT_H - 1))
        memb_t = T(pp, [E * M, EMB], FP32, "memb")
        nc.vector.tensor_add(memb_t[:], ps_memb[:], btrans_bc[0:E * M, :])
        nc.vector.tensor_copy(nodes_m[:, 0:EMB], memb_t[:])
        ememb_t = T(pp, [E * M, EMB], FP32, "ememb")
        nc.scalar.activation(ememb_t[:], memb_t[:], ACT.Exp)
        ps_ent = T(pps, [E, EMB], FP32, "ps")
        nc.tensor.matmul(ps_ent[:], g3_t, ememb_t[:], start=True, stop=True)
        nc.scalar.activation(nodes_e[:, 0:EMB], ps_ent[:], ACT.Ln)
        chaff(6)

        # ---- S3: link nodes (scale folded into gspan, not xspan) ----
        gsc_t = []
        for i in range(SPAN_T):
            a = T(pst, [128, 1], FP32, "aT", bufs=4)
            nc.vector.tensor_reduce(a[:], attl_t[:, i * 192:(i + 1) * 192],
                                    mybir.AxisListType.X, mybir.AluOpType.add)
            g = T(pp, [128, L], SEQ_DT, f"gsc{i}")
            nc.vector.tensor_scalar_mul(g[:], gspan_t[:, i * L:(i + 1) * L], a[:])
            gsc_t.append(g)
        ps_as = T(pps, [L, 1], FP32, "ps")
        for i in range(SPAN_T):
            nc.tensor.matmul(ps_as[:], gsc_t[i][:], onesb_t,
                             start=(i == 0), stop=(i == SPAN_T - 1))
        asum_t = T(pp, [L, 1], FP32, "asum")
        nc.vector.tensor_scalar_mul(asum_t[:], ps_as[:], 1.0 / (NH * LS))
        ps_lct = [T(pps, [128, L], FP32, "ps") for _ in range(KT_H)]
        for i in range(SPAN_T):
            for mt in range(KT_H):
                nc.tensor.matmul(ps_lct[mt][:],
                                 xspan_t[:, i * H + mt * 128:i * H + (mt + 1) * 128],
                                 gsc_t[i][:],
                                 start=(i == 0), stop=(i == SPAN_T - 1))
        lct_t = []
        for mt in range(KT_H):
            t = T(pp, [128, L], SEQ_DT, f"lct{mt}")
            nc.vector.tensor_scalar_mul(t[:], ps_lct[mt][:], 1.0 / (NH * LS))
            lct_t.append(t)
        bterm_t = T(pp, [L, EMB], FP32, "bterm")
        nc.vector.tensor_scalar_mul(bterm_t[:], btrans_bc[0:L, :], asum_t[:])
        ps_link = T(pps, [L, EMB], FP32, "ps")
        for kt in range(KT_H):
            nc.tensor.matmul(ps_link[:], lct_t[kt][:],
                             wtrans_t[:, kt * EMB:(kt + 1) * EMB],
                             start=(kt == 0), stop=(kt == KT_H - 1))
        nc.vector.tensor_add(nodes_l[:, 0:EMB], ps_link[:], bterm_t[:])
        chaff(6)

        # rsum[e] = sum_rows gmat[row,e] * rowsum(attm[row,:]) — first half
        arsum_t = T(pp, [128, ATTM_T], FP32, "arsum")
        for i in range(4):
            nc.vector.tensor_reduce(arsum_t[:, i:i + 1], attm_t[:, i * C:(i + 1) * C],
                                    mybir.AxisListType.X, mybir.AluOpType.add)
        # adjacency row-normalize (entity destination columns only)
        ps_rsE = T(pps, [1, 5 * E], FP32, "ps")
        for gi, (goff, gsz) in enumerate(NODE_GROUPS):
            nc.tensor.matmul(ps_rsE[:], onesb_t[0:gsz, 0:1], adjf_t[gi],
                             start=(gi == 0), stop=(gi == 2))
        rs_t = T(pp, [1, 5 * E], FP32, "rs")
        nc.vector.tensor_scalar_add(rs_t[:], ps_rsE[:], 1e-5)
        rcp_t = T(pp, [1, 5 * E], FP32, "rcp")
        nc.vector.reciprocal(rcp_t[:], rs_t[:])
        rsbc_t = T(pp, [128, 5 * E], FP32, "rsbc")
        nc.gpsimd.partition_broadcast(rsbc_t[:], rcp_t[:])
        nc.gpsimd.dma_start(w1c_t[2][:], w1[:, 12800:19200])
        rsbcb_t = T(pp, [128, 5 * E], GRAPH_DT, "rsbcb")
        nc.vector.tensor_copy(rsbcb_t[:], rsbc_t[:])
        adjn_t = []
        for gi, (goff, gsz) in enumerate(NODE_GROUPS):
            t = T(pp, [gsz, 5 * E], GRAPH_DT, f"adjn{gi}")
            nc.vector.tensor_mul(t[:], adjf_t[gi], rsbcb_t[0:gsz, :])
            adjn_t.append(t)

        msgE_t = []
        for i in range(5):
            ksz = 128 if i < 4 else 20
            ps = T(pps, [ksz, 5 * E], FP32, "ps")
            for gi, (goff, gsz) in enumerate(NODE_GROUPS):
                nc.tensor.matmul(ps[:], node_tiles[gi][:, i * 128:i * 128 + ksz],
                                 adjn_t[gi][:],
                                 start=(gi == 0), stop=(gi == 2))
            t = T(pp, [ksz, 5 * E], GRAPH_DT, f"msgE{i}")
            nc.vector.tensor_copy(t[:], ps[:])
            msgE_t.append(t)

        # ---- S4: eaT directly transposed; e_ctx = (ean@X)@W + rn*b ----
        for i in range(4, ATTM_T):
            nc.vector.tensor_reduce(arsum_t[:, i:i + 1], attm_t[:, i * C:(i + 1) * C],
                                    mybir.AxisListType.X, mybir.AluOpType.add)
        arsumb_t = T(pp, [128, ATTM_T], GRAPH_DT, "arsumb")
        nc.vector.tensor_copy(arsumb_t[:], arsum_t[:])
        ps_ct = [T(pps, [128, E], FP32, "ps") for _ in range(8)]
        for i in range(4):
            for ct in range(8):
                nc.tensor.matmul(ps_ct[ct][:],
                                 attm_t[:, i * C + ct * 128:i * C + (ct + 1) * 128],
                                 gmat_t[:, i * E:(i + 1) * E],
                                 start=(i == 0), stop=(i == 3))
        eaP_t = []
        for ct in range(8):
            t = T(pst, [128, E], FP32, "eaP", bufs=8)
            nc.vector.tensor_copy(t[:], ps_ct[ct][:])
            eaP_t.append(t)
        chaff(96)
        ps_ct2 = [T(pps, [128, E], FP32, "ps") for _ in range(8)]
        for i in range(4, ATTM_T):
            for ct in range(8):
                nc.tensor.matmul(ps_ct2[ct][:],
                                 attm_t[:, i * C + ct * 128:i * C + (ct + 1) * 128],
                                 gmat_t[:, i * E:(i + 1) * E],
                                 start=(i == 4), stop=(i == ATTM_T - 1))
        eaTb_t = []
        for ct in range(8):
            t = T(pp, [128, E], GRAPH_DT, f"eaTb{ct}")
            nc.vector.tensor_add(t[:], ps_ct2[ct][:], eaP_t[ct][:])
            eaTb_t.append(t)
        ps_rs = T(pps, [E, 1], FP32, "ps")
        for i in range(ATTM_T):
            nc.tensor.matmul(ps_rs[:], gmat_t[:, i * E:(i + 1) * E],
                             arsumb_t[:, i:i + 1],
                             start=(i == 0), stop=(i == ATTM_T - 1))
        rsum_t = T(pp, [E, 1], FP32, "rsum")
        nc.vector.tensor_scalar_add(rsum_t[:], ps_rs[:], 1e-5)
        recip_t = T(pp, [E, 1], FP32, "recip")
        nc.vector.reciprocal(recip_t[:], rsum_t[:])
        # rn = rsum/(rsum+eps) = 1 - eps*recip
        # recip as a row vector broadcast across partitions
        ps_rt = T(pps, [1, E], FP32, "ps")
        nc.tensor.transpose(ps_rt[:], recip_t[:], f32p_t[0:E, 22:44])
        recipT_t = T(pp, [1, E], FP32, "recipT")
        nc.vector.tensor_copy(recipT_t[:], ps_rt[:])
        recipT_bc = T(pp, [128, E], FP32, "recipT_bc")
        nc.gpsimd.partition_broadcast(recipT_bc[:], recipT_t[:])

        # eanXT[h,e] tiles directly (no transposes), normalized per entity
        ps_xT = [T(pps, [128, E], FP32, "ps") for _ in range(KT_H)]
        for ct in range(8):
            for ht in range(KT_H):
                nc.tensor.matmul(ps_xT[ht][:],
                                 xfull_t[:, ct * H + ht * 128:ct * H + (ht + 1) * 128],
                                 eaTb_t[ct][:], start=(ct == 0), stop=(ct == 7))
        eanXT_t = []
        for ht in range(KT_H):
            t = T(pp, [128, E], SEQ_DT, f"eanXT{ht}")
            nc.vector.tensor_mul(t[:], ps_xT[ht][:], recipT_bc[:])
            eanXT_t.append(t)
        chaff(6)
        ps_ectxT = [T(pps, [128, E], FP32, "ps") for _ in range(4)]
        for ht in range(KT_H):
            for mt in range(4):
                nc.tensor.matmul(ps_ectxT[mt][:],
                                 wtrans_t[:, ht * EMB + mt * 128:ht * EMB + (mt + 1) * 128],
                                 eanXT_t[ht][:], start=(ht == 0), stop=(ht == KT_H - 1))
        # rnT = rowsum(ean) as a row = 1 - eps*recipT
        rnT_t = T(pp, [1, E], FP32, "rnT")
        nc.vector.tensor_scalar_mul(rnT_t[:], recipT_t[:], -1e-5)
        nc.vector.tensor_scalar_add(rnT_t[:], rnT_t[:], 1.0)
        rnT_bc = T(pp, [128, E], FP32, "rnT_bc")
        nc.gpsimd.partition_broadcast(rnT_bc[:], rnT_t[:])
        ectxT_t = []
        for mt in range(4):
            bt_ = T(pst, [128, E], FP32, "ebias", bufs=2)
            nc.vector.tensor_scalar_mul(bt_[:], rnT_bc[:], f32p_t[:, 70 + mt:71 + mt])
            t = T(pp, [128, E], PAIR_DT, f"ectxT{mt}")
            nc.vector.tensor_add(t[:], ps_ectxT[mt][:], bt_[:])
            ectxT_t.append(t)
        ectxb_t = T(pp, [E, EMB], PAIR_DT, "ectxb")
        for mt in range(4):
            ps = T(pps, [E, 64], FP32, "ps")
            psb = ps[:].bitcast(PAIR_DT)
            nc.tensor.transpose(psb, ectxT_t[mt][:], bf16p_t[:, 0:128])
            nc.vector.tensor_copy(ectxb_t[:, mt * 128:(mt + 1) * 128], psb)

        # ---- S5: RGCN (entity rows only) ----
        ps_gcnT = [T(pps, [128, E], FP32, "ps") for _ in range(4)]
        term = 0
        for r in range(5):
            for i in range(4):
                for mt in range(4):
                    nc.tensor.matmul(
                        ps_gcnT[mt][:],
                        wrel_t[:, (r * 4 + i) * EMB + mt * 128:(r * 4 + i) * EMB + (mt + 1) * 128],
                        msgE_t[i][:, r * E:(r + 1) * E],
                        start=(term == 0), stop=False)
                term += 1
        for r in range(5):
            for mt in range(4):
                nc.tensor.matmul(ps_gcnT[mt][:],
                                 wrels_t[0:20, r * EMB + mt * 128:r * EMB + (mt + 1) * 128],
                                 msgE_t[4][0:20, r * E:(r + 1) * E],
                                 start=False, stop=(r == 4))
        nc.gpsimd.dma_start(w1c_t[3][:], w1[:, 19200:25600])
        gcnT_t = []
        for mt in range(4):
            t = T(pp, [128, E], PAIR_DT, f"gcnT{mt}")
            nc.scalar.activation(t[:], ps_gcnT[mt][:], ACT.Relu,
                                 bias=f32p_t[:, 66 + mt:67 + mt])
            gcnT_t.append(t)
        entb_t = T(pp, [E, EMB], PAIR_DT, "entb")
        for mt in range(4):
            ps = T(pps, [E, 64], FP32, "ps")
            psb = ps[:].bitcast(PAIR_DT)
            nc.tensor.transpose(psb, gcnT_t[mt][:], bf16p_t[:, 0:128])
            nc.vector.tensor_copy(entb_t[:, mt * 128:(mt + 1) * 128], psb)

        # ---- S6: relation map x (bf16 transposes + row selection) ----
        ps_c1 = [T(pps, [128, R1 * S], FP32, "ps") for _ in range(2)]
        entT_t, ectxTv_t = gcnT_t, ectxT_t
        entS_t, ectxS_t = [], []
        for mt in range(4):
            for src_t, dst_list, nf in ((entb_t, entS_t, "entS"),
                                        (ectxb_t, ectxS_t, "ectxS")):
                ps = T(pps, [128, 17], FP32, "ps")
                nc.tensor.matmul(ps[:], src_t[:, mt * 128:(mt + 1) * 128], psel_t,
                                 start=True, stop=True)
                t = T(pp, [128, 17], PAIR_DT, f"{nf}{mt}")
                nc.vector.tensor_copy(t[:], ps[:])
                dst_list.append(t)
            t1 = T(pst, [128, 17 * S], FP32, "xtmp", bufs=2)
            nc.vector.tensor_mul(
                t1[:].rearrange("p (a b) -> p a b", a=17, b=S),
                entS_t[mt][:].unsqueeze(2).to_broadcast((128, 17, S)),
                entT_t[mt][:].unsqueeze(1).to_broadcast((128, 17, S)))
            t2 = T(pst, [128, 17 * S], FP32, "xtmp", bufs=2)
            nc.vector.tensor_mul(
                t2[:].rearrange("p (a b) -> p a b", a=17, b=S),
                ectxS_t[mt][:].unsqueeze(2).to_broadcast((128, 17, S)),
                ectxTv_t[mt][:].unsqueeze(1).to_broadcast((128, 17, S)))
            inner = xpad_t[mt][:].rearrange("p (a b) -> p a b", a=BIN, b=PW)[
                :, 2:2 + 17, 2:2 + S]
            nc.vector.tensor_add(inner,
                                 t1[:].rearrange("p (a b) -> p a b", a=17, b=S),
                                 t2[:].rearrange("p (a b) -> p a b", a=17, b=S))
            if mt == 0:
                chaff(16)

        # conv1: 512 -> 256
        for kt in range(4):
            w = w1c_t[kt]
            for tap in range(25):
                di, dj = divmod(tap, 5)
                rhs = xpad_t[kt][:].rearrange("p (a b) -> p a b", a=BIN, b=PW)[
                    :, di:di + R1, dj:dj + S]
                for mt in range(2):
                    nc.tensor.matmul(ps_c1[mt][:],
                                     w[:, tap * 256 + mt * 128:tap * 256 + (mt + 1) * 128],
                                     rhs, start=(kt == 0 and tap == 0),
                                     stop=(kt == 3 and tap == 24))
        for mt in range(2):
            inner = pad1_t[mt][:].rearrange("p (a b) -> p a b", a=B1R, b=PW)[
                :, 2:2 + R1, 2:2 + S]
            nc.scalar.activation(inner,
                                 ps_c1[mt][:].rearrange("p (a b) -> p a b", a=R1, b=S),
                                 ACT.Relu, bias=biasp_t[:, mt:mt + 1])

        # conv2: 256 -> 256
        ps_c2 = [T(pps, [128, R2 * S], FP32, "ps") for _ in range(2)]
        for kt in range(2):
            w = T(pst, [128, 6400], CONV_DT, "wconv", bufs=4)
            dma(w[:], w2[:, kt * 6400:(kt + 1) * 6400])
            for tap in range(25):
                di, dj = divmod(tap, 5)
                rhs = pad1_t[kt][:].rearrange("p (a b) -> p a b", a=B1R, b=PW)[
                    :, di:di + R2, dj:dj + S]
                for mt in range(2):
                    nc.tensor.matmul(ps_c2[mt][:],
                                     w[:, tap * 256 + mt * 128:tap * 256 + (mt + 1) * 128],
                                     rhs, start=(kt == 0 and tap == 0),
                                     stop=(kt == 1 and tap == 24))
        for mt in range(2):
            inner = pad2_t[mt][:].rearrange("p (a b) -> p a b", a=B2R, b=PW)[
                :, 2:2 + R2, 2:2 + S]
            nc.scalar.activation(inner,
                                 ps_c2[mt][:].rearrange("p (a b) -> p a b", a=R2, b=S),
                                 ACT.Relu, bias=biasp_t[:, 2 + mt:3 + mt])

        # wht streamed now so its 4.2MB overlaps conv2/conv3 compute
        whtc = []
        for c in range(2):
            t = T(pst, [128, 8 * 1024], PAIR_DT, "bigreuse", bufs=2)
            dma(t[:], wht[:, c * 8192:(c + 1) * 8192])
            whtc.append(t)

        # retire chaff psum so it isn't dead code
        warm_sb = T(pp, [128, 128], FP32, "warm_sb")
        nc.vector.tensor_copy(warm_sb[:], chaff_ps[:])
        warm_dram = pdram.tile([128, 128], FP32, name="warm_dram")
        dma(warm_dram[:], warm_sb[:])

        # conv3: 256 -> 512, four (kt, taphalf) chunks
        x3_t = [T(pp, [128, SP2], PAIR_DT, f"x3_{mt}") for mt in range(4)]
        ps_c3 = [T(pps, [128, SP2], FP32, "ps") for _ in range(4)]
        for kt in range(2):
            for taps in (range(0, 13), range(13, 25)):
                w = T(pst, [128, len(taps) * EMB], CONV_DT, "wconv", bufs=4)
                dma(w[:], w3[:, (kt * 25 + taps.start) * EMB:(kt * 25 + taps.stop) * EMB])
                for tj, tap in enumerate(taps):
                    di, dj = divmod(tap, 5)
                    rhs = pad2_t[kt][:].rearrange("p (a b) -> p a b", a=B2R, b=PW)[
                        :, di:di + R3, dj:dj + S]
                    for mt in range(4):
                        nc.tensor.matmul(ps_c3[mt][:],
                                         w[:, tj * EMB + mt * 128:tj * EMB + (mt + 1) * 128],
                                         rhs, start=(kt == 0 and tap == 0),
                                         stop=(kt == 1 and tap == 24))
        for mt in range(4):
            nc.scalar.activation(x3_t[mt][:], ps_c3[mt][:], ACT.Relu,
                                 bias=biasp_t[:, 4 + mt:5 + mt])

        # ---- S7: pair features + classifier ----
        SP_TILES = [(0, 128), (128, SP2 - 128)]
        x3T_t = [T(pp, [sz, EMB], PAIR_DT, f"x3T{i}")
                 for i, (off, sz) in enumerate(SP_TILES)]
        for i, (off, sz) in enumerate(SP_TILES):
            for src in range(4):
                ps = T(pps, [sz, 64], FP32, "ps")
                psb = ps[:].bitcast(PAIR_DT)
                nc.tensor.transpose(psb, x3_t[src][:, off:off + sz], bf16p_t[:, 0:128])
                nc.vector.tensor_copy(x3T_t[i][:, src * 128:(src + 1) * 128], psb)
        chaff(20)

        shst_t = T(pp, [E, 2 * PH2], PAIR_DT, "shst")
        dma(shst_t[:], shst)
        smp_t = T(pp, [128, 2 * PH2], PAIR_DT, "smp")
        dma(smp_t[:], smp)

        featT = [None] * 16
        for src in range(2):
            for mt in range(4):
                ps = T(pps, [128, PH2], FP32, "ps")
                nc.tensor.matmul(ps[:], entb_t[:, mt * 128:(mt + 1) * 128],
                                 shst_t[:, src * PH2:(src + 1) * PH2],
                                 start=True, stop=True)
                t = T(pp, [128, PH2], PAIR_DT, f"featT{4 * src + mt}")
                nc.vector.tensor_copy(t[:], ps[:])
                featT[4 * src + mt] = t
        for mt in range(4):
            ps = T(pps, [128, PH2], FP32, "ps")
            for i, (off, sz) in enumerate(SP_TILES):
                nc.tensor.matmul(ps[:], x3T_t[i][0:sz, mt * 128:(mt + 1) * 128],
                                 smp_t[0:sz, i * PH2:(i + 1) * PH2],
                                 start=(i == 0), stop=(i == 1))
            t = T(pp, [128, PH2], PAIR_DT, f"featT{8 + mt}")
            nc.vector.tensor_copy(t[:], ps[:])
            featT[8 + mt] = t
        for mt in range(4):
            t = T(pp, [128, PH2], PAIR_DT, f"featT{12 + mt}")
            nc.vector.tensor_mul(t[:], featT[mt][:], featT[4 + mt][:])
            featT[12 + mt] = t

        chaff(12)
        ps_ht = [T(pps, [128, PH2], FP32, "ps") for _ in range(8)]
        for kt in range(16):
            wv = whtc[kt // 8]
            for mt in range(8):
                nc.tensor.matmul(
                    ps_ht[mt][:],
                    wv[:, (kt % 8) * 1024 + mt * 128:(kt % 8) * 1024 + (mt + 1) * 128],
                    featT[kt][:], start=(kt == 0), stop=(kt == 15))
        htT_t = []
        for mt in range(8):
            t = T(pp, [128, PH2], PAIR_DT, f"htT{mt}")
            nc.scalar.activation(t[:], ps_ht[mt][:], ACT.Tanh,
                                 bias=biasp_t[:, 12 + mt:13 + mt])
            htT_t.append(t)

        chaff(6)
        wbil_t = T(pp, [128, 8 * 97], PAIR_DT, "wbil")
        dma(wbil_t[:], wbil)
        ps_out = T(pps, [97, PH2], FP32, "ps")
        for kt in range(8):
            nc.tensor.matmul(ps_out[:], wbil_t[:, kt * 97:(kt + 1) * 97],
                             htT_t[kt][:], start=(kt == 0), stop=(kt == 7))
        out_t = T(pp, [97, PH2], FP32, "out")
        nc.vector.tensor_scalar_add(out_t[:], ps_out[:], biasp_t[0:97, 20:21])
        dma(outt, out_t[:])

    nc.compile()
    return nc


_PROG = None


def _get_prog():
    global _PROG
    if _PROG is None:
        _PROG = build_program()
    return _PROG


def _np(dt):
    return _NPDT[dt]


def _pack_rows(a, ntiles, dt):
    """[ntiles*128, W] (zero-padded) -> [128, ntiles*W] with tile i at
    column block i."""
    r, w = a.shape
    pad = ntiles * 128 - r
    if pad:
        a = np.concatenate([a, np.zeros((pad, w), a.dtype)], axis=0)
    return np.ascontiguousarray(
        a.reshape(ntiles, 128, w).transpose(1, 0, 2).reshape(128, ntiles * w),
        _np(dt))


def _pack_conv(w, flip, dt):
    """conv weight OIHW -> tap-major per-kt chunks [128, ...]."""
    w = np.asarray(w, np.float32)
    if flip:
        w = w[:, :, ::-1, :]
    oc, ic, _, _ = w.shape
    t = w.transpose(2, 3, 1, 0).reshape(25, ic, oc)   # (tap, ic, oc)
    nkt = ic // 128
    chunks = [np.ascontiguousarray(
        t[:, kt * 128:(kt + 1) * 128, :].transpose(1, 0, 2).reshape(128, 25 * oc))
        for kt in range(nkt)]
    return np.ascontiguousarray(np.concatenate(chunks, axis=1), _np(dt))


def _pack_conv3(w, flip, dt):
    """conv3 weights as (kt, taphalf) chunks: [128, 25*512] per kt with
    taps in order — column block (kt*25 + tap)*512."""
    w = np.asarray(w, np.float32)
    if flip:
        w = w[:, :, ::-1, :]
    t = w.transpose(2, 3, 1, 0).reshape(25, IC, EMB)
    chunks = [np.ascontiguousarray(
        t[:, kt * 128:(kt + 1) * 128, :].transpose(1, 0, 2).reshape(128, 25 * EMB))
        for kt in range(2)]
    return np.ascontiguousarray(np.concatenate(chunks, axis=1), _np(dt))


def _shared_inputs(inputs):
    f32 = np.float32
    sh = {}
    fp = np.zeros((128, 74), f32)
    fp[:, 0] = np.asarray(inputs["conv1_b"], f32)[0:128]
    fp[:, 1] = np.asarray(inputs["conv1_b"], f32)[128:256]
    fp[:, 2] = np.asarray(inputs["conv2_b"], f32)[0:128]
    fp[:, 3] = np.asarray(inputs["conv2_b"], f32)[128:256]
    for mt in range(4):
        fp[:, 4 + mt] = np.asarray(inputs["conv3_b"], f32)[mt * 128:(mt + 1) * 128]
    for mt in range(8):
        fp[:, 12 + mt] = np.asarray(inputs["ht_b"], f32)[mt * 128:(mt + 1) * 128]
    fp[0:97, 20] = np.asarray(inputs["bil_b"], f32)
    fp[:, 21] = 1.0
    fp[0:22, 22:44] = np.eye(22, dtype=f32)
    fp[0:E * M, 44:66] = np.kron(np.eye(E, dtype=f32), np.ones((M, 1), f32))
    for mt in range(4):
        fp[:, 66 + mt] = np.asarray(inputs["b_rgcn"], f32)[mt * 128:(mt + 1) * 128]
        fp[:, 70 + mt] = np.asarray(inputs["b_trans"], f32)[mt * 128:(mt + 1) * 128]
    sh["f32p"] = fp
    bb = np.zeros((1, 1024), f32)
    bb[0, 0:512] = np.asarray(inputs["b_trans"], f32)
    bb[0, 512:1024] = np.asarray(inputs["b_rgcn"], f32)
    sh["btb"] = bb
    bt = np.zeros((128, 1206), np.float32)
    bt[:, 0:128] = np.eye(128, dtype=f32)
    bt[:, 524:644] = _pack_rows(np.kron(np.eye(L, dtype=f32),
                                        np.ones((LS, 1), f32)), SPAN_T, FP32)
    bt[:, 644:798] = _pack_rows(np.kron(np.eye(E, dtype=f32),
                                        np.ones((M * NH, 1), f32) / (M * NH)),
                                ATTM_T, FP32)
    bt[:, 1205] = 1.0
    sh["bf16p_base"] = bt
    sh["wtrans"] = _pack_rows(np.asarray(inputs["W_trans"], f32), KT_H, SEQ_DT)
    # wrel: 5 relations (4 + self); big k-tiles [128] and the 20-row tail
    wr = np.zeros((5, 532, EMB), f32)
    wr[0:4] = np.asarray(inputs["W_rel"], f32)
    wr[4] = np.asarray(inputs["W_self"], f32)
    wrb = wr[:, 0:512, :].reshape(5, 4, 128, EMB)   # (r, i, 128, 512)
    sh["wrel"] = np.ascontiguousarray(
        wrb.transpose(2, 0, 1, 3).reshape(128, 20 * EMB), _np(GRAPH_DT))
    sh["wrels"] = np.ascontiguousarray(
        wr[:, 512:532, :].transpose(1, 0, 2).reshape(20, 5 * EMB), _np(GRAPH_DT))
    sh["w1"] = [_pack_conv(inputs["conv1_w"], fl, CONV_DT) for fl in (0, 1)]
    sh["w2"] = [_pack_conv(inputs["conv2_w"], fl, CONV_DT) for fl in (0, 1)]
    sh["w3"] = [_pack_conv3(inputs["conv3_w"], fl, CONV_DT) for fl in (0, 1)]
    sh["wht"] = _pack_rows(np.asarray(inputs["ht_W"], f32), 16, PAIR_DT)
    sh["wbil"] = _pack_rows(np.asarray(inputs["bil_W"], f32), 8, PAIR_DT)
    psel = []
    for fl in (0, 1):
        pm = np.zeros((E, 17), f32)
        for r in range(17):
            pm[(21 - r) if fl else r, r] = 1.0
        psel.append(pm)
    sh["psel"] = psel
    return sh


def _pair_idx(hts_b, hh):
    h = np.asarray(hts_b)[:, 0]
    mask = (h <= 10) if hh == 0 else (h >= 11)
    idx = np.nonzero(mask)[0]
    if len(idx) > PH2:
        raise RuntimeError(f"pair overflow: {len(idx)} > {PH2}")
    return idx


def _core_inputs(inputs, shared, b, hh):
    f32 = np.float32
    X = np.asarray(inputs["sequence_output"][b], f32)
    att = np.asarray(inputs["attention"][b], f32)
    adj = np.asarray(inputs["adjacency"][b], f32)
    mf = np.asarray(inputs["mention_idx"][b]).reshape(-1).astype(np.int64)
    ls = np.asarray(inputs["link_start"][b]).reshape(-1).astype(np.int64)
    ntypes = np.asarray(inputs["node_types"][b]).astype(np.int64)
    hts = np.asarray(inputs["hts"][b]).astype(np.int64)

    m = {k: shared[k] for k in
         ("f32p", "btb", "wtrans", "wrel", "wrels", "wht", "wbil")}
    m["w1"] = shared["w1"][hh]
    m["w2"] = shared["w2"][hh]
    m["w3"] = shared["w3"][hh]
    bt = shared["bf16p_base"].copy()
    bt[:, 128:524] = _pack_rows(np.ascontiguousarray(X[mf].T), KT_H, FP32)
    bt[0:E, 798:815] = shared["psel"][hh]
    te = np.asarray(inputs["type_embed"], f32)[ntypes]
    adjc = np.concatenate([adj[r].T[:, 0:E] for r in range(4)]
                          + [np.eye(NN, E, dtype=f32)], axis=1)
    for gi, (goff, gsz) in enumerate([(0, E), (E, E * M), (E + E * M, L)]):
        bt[0:gsz, 815 + gi * 20:835 + gi * 20] = te[goff:goff + gsz]
        bt[0:gsz, 875 + gi * 110:985 + gi * 110] = adjc[goff:goff + gsz]
    m["bf16p"] = np.ascontiguousarray(bt, _np(PAIR_DT))
    pos = ls[:, None] + np.arange(LS)
    m["xspan"] = _pack_rows(X[pos.reshape(-1)], SPAN_T, SEQ_DT)
    m["xfull"] = _pack_rows(X, 8, SEQ_DT)
    rows = att[:, mf, :]
    m["attm"] = _pack_rows(rows.transpose(1, 0, 2).reshape(E * M * NH, C),
                           ATTM_T, GRAPH_DT)
    attl = np.empty((L * LS, NH * LS), f32)
    for l in range(L):
        blk = att[:, pos[l], :][:, :, pos[l]]
        attl[l * LS:(l + 1) * LS, :] = blk.transpose(2, 0, 1).reshape(LS, NH * LS)
    m["attl"] = _pack_rows(attl, SPAN_T, GRAPH_DT)

    idx = _pair_idx(hts, hh)
    pr = hts[idx]
    n = len(idx)
    shm = np.zeros((E, 2 * PH2), f32)
    shm[pr[:, 0], np.arange(n)] = 1.0
    shm[pr[:, 1], PH2 + np.arange(n)] = 1.0
    m["shst"] = np.ascontiguousarray(shm, _np(PAIR_DT))
    loc_r = pr[:, 0] if hh == 0 else (21 - pr[:, 0])
    smm = np.zeros((SP2, PH2), f32)
    smm[loc_r * S + pr[:, 1], np.arange(n)] = 1.0
    m["smp"] = _pack_rows(smm, 2, PAIR_DT)
    return m


def kernel(**inputs):
    nc = _get_prog()
    shared = _shared_inputs(inputs)
    in_maps = []
    for b in range(B):
        for hh in range(2):
            in_maps.append(_core_inputs(inputs, shared, b, hh))
    res = run_bass_kernel_spmd(nc, in_maps, core_ids=list(range(8)))
    out = np.empty((B, P, 97), np.float32)
    for b in range(B):
        for hh in range(2):
            idx = _pair_idx(np.asarray(inputs["hts"][b]), hh)
            r = np.asarray(res.results[2 * b + hh]["outt"], np.float32)
            out[b, idx, :] = r[:, 0:len(idx)].T
    return out


# revision 33
# speedup vs baseline: 1.0275x; 1.0161x over previous
"""Trainium2 Bass kernel for nn_DocREModel (8-core SPMD).

Sharding: data-parallel over the 4 documents x 2 halves = 8 cores.
Each doc's two cores duplicate the (cheap) graph phase, then split the
expensive conv reasoning stack SPATIALLY: core A computes output rows
0..10 of the 22x22 relation map, core B rows 11..21.  B works on a
row-FLIPPED frame with row-flipped conv taps so both cores run the
identical instruction stream (SPMD) -- only the data differs.  Pairs
are split by head entity (h<=10 vs h>=11), padded to PH2=288 columns.

e_ctx is reassociated:  ean @ (X @ W + b) == (ean @ X) @ W + rowsum(ean)*b,
which removes the full [1024,512] seq transform.

All floating-point arithmetic runs on device.  Host does only
index-driven data movement: batch slicing, transposes, row gathers at
integer indices, one-hot/selector construction, and layout packing.

DMA discipline: every sizable tensor is packed host-side as [128, W]
with per-partition-contiguous rows so each tensor is ONE dma_start
(the SP sequencer pays ~565ns per dma_start; the baseline's 334 DMAs
serialized 205us of sync time).
"""

import numpy as np
from contextlib import ExitStack

import concourse.bass as bass
import concourse.bacc as bacc
import concourse.tile as tile
import concourse.mybir as mybir
from concourse.bass_utils import run_bass_kernel_spmd

import ml_dtypes

FP32 = mybir.dt.float32
BF16 = mybir.dt.bfloat16

SEQ_DT = BF16
CONV_DT = BF16
PAIR_DT = BF16
GRAPH_DT = BF16

_NPDT = {FP32: np.float32, BF16: ml_dtypes.bfloat16}

B, C, H, NH = 4, 1024, 768, 12
E, M, L, LS = 22, 3, 30, 16
NN, EMB = 118, 512
P = 462
IC = 256
S = 22              # spatial side of relation map
PW = 26             # padded column width (2+22+2)
R1, R2, R3 = 15, 13, 11   # conv1/2/3 computed rows (half + halo)
BIN, B1R, B2R = 21, 19, 17  # padded row counts of conv input/1/2 buffers
SP2 = R3 * S        # 242 flattened conv3 output
PH2 = 288           # padded pairs per core (h-split of 462)
ACT = mybir.ActivationFunctionType
KT_H = H // 128     # 6
ATTM_T = 7          # ceil(792/128)
SPAN_T = 4          # ceil(480/128)


def build_program():
    nc = bacc.Bacc("TRN2", target_bir_lowering=False, debug=False)

    dins = {}

    def din(name, shape, dt=FP32):
        dins[name] = nc.dram_tensor(name, shape, dt, kind="ExternalInput").ap()
        return dins[name]

    f32p = din("f32p", [128, 74], FP32)       # biasp|ident22|g3|brgcnT|btransT
    btb = din("btb", [1, 1024], FP32)         # b_trans|b_rgcn
    bf16p = din("bf16p", [128, 1206], PAIR_DT)  # identp|xg|gspan|gmat|psel|typ|adjf|ones
    wtrans = din("wtrans", [128, KT_H * EMB], SEQ_DT)
    attl = din("attl", [128, SPAN_T * NH * LS], GRAPH_DT)
    xspan = din("xspan", [128, SPAN_T * H], SEQ_DT)
    attm = din("attm", [128, ATTM_T * C], GRAPH_DT)
    xfull = din("xfull", [128, 8 * H], SEQ_DT)
    wrel = din("wrel", [128, 20 * EMB], GRAPH_DT)  # (r,i<4)-tile at col (r*4+i)*512
    wrels = din("wrels", [20, 5 * EMB], GRAPH_DT)  # the 20-row k-tiles, per relation
    w1 = din("w1", [128, 25600], CONV_DT)     # 4 kt-chunks of [128, 25*256]
    w2 = din("w2", [128, 12800], CONV_DT)     # 2 kt-chunks
    w3 = din("w3", [128, 25600], CONV_DT)     # (kt, taphalf) chunks
    shst = din("shst", [E, 2 * PH2], PAIR_DT)
    smp = din("smp", [128, 2 * PH2], PAIR_DT)  # 2 row-tiles of sm
    wht = din("wht", [128, 16 * 1024], PAIR_DT)
    wbil = din("wbil", [128, 8 * 97], PAIR_DT)
    outt = nc.dram_tensor("outt", [97, PH2], FP32, kind="ExternalOutput").ap()

    with tile.TileContext(nc) as tc, ExitStack() as ctx:
        pp = ctx.enter_context(tc.tile_pool(name="persist", bufs=1))
        pst = ctx.enter_context(tc.tile_pool(name="stream", bufs=1))
        pps = ctx.enter_context(tc.tile_pool(name="psum", bufs=8, space="PSUM"))
        pdram = ctx.enter_context(tc.tile_pool(name="dram", bufs=1, space="DRAM"))

        dma = nc.sync.dma_start

        def T(pool, shape, dt, tag, bufs=None):
            return pool.tile(shape, dt, tag=tag, name=tag, bufs=bufs)

        # ---- persistent tiles; two packed small-tensor DMAs + big tensors ----
        bf16p_t = T(pp, [128, 1206], PAIR_DT, "bf16p")
        dma(bf16p_t[:], bf16p)
        f32p_t = T(pp, [128, 74], FP32, "f32p")
        dma(f32p_t[:], f32p)
        btb_t = T(pp, [1, 1024], FP32, "btb")
        dma(btb_t[:], btb)
        identp_t = bf16p_t[:, 0:128]
        xg_t = bf16p_t[:, 128:524]
        gspan_t = bf16p_t[:, 524:644]
        gmat_t = bf16p_t[:, 644:798]
        psel_t = bf16p_t[0:E, 798:815]
        TYP_OFF, ADJ_OFF = 815, 875
        onesb_t = bf16p_t[:, 1205:1206]
        biasp_t = f32p_t[:, 0:22]
        g3_t = f32p_t[0:E * M, 44:66]
        wtrans_t = T(pp, [128, KT_H * EMB], SEQ_DT, "wtrans")
        dma(wtrans_t[:], wtrans)
        attl_t = T(pp, [128, SPAN_T * NH * LS], GRAPH_DT, "attl")
        dma(attl_t[:], attl)
        attm_t = T(pst, [128, ATTM_T * C], GRAPH_DT, "bigreuse", bufs=2)
        dma(attm_t[:, 0:4 * C], attm[:, 0:4 * C])
        xspan_t = T(pp, [128, SPAN_T * H], SEQ_DT, "xspan")
        dma(xspan_t[:], xspan)
        shst_t = T(pp, [E, 2 * PH2], PAIR_DT, "shst")
        dma(shst_t[:], shst)
        dma(attm_t[:, 4 * C:ATTM_T * C], attm[:, 4 * C:ATTM_T * C])
        xfull_t = T(pst, [128, 8 * H], SEQ_DT, "bigreuse", bufs=2)
        dma(xfull_t[:, 0:4 * H], xfull[:, 0:4 * H])
        dma(xfull_t[:, 4 * H:8 * H], xfull[:, 4 * H:8 * H])
        wrel_t = T(pp, [128, 20 * EMB], GRAPH_DT, "wrel")
        wrels_t = T(pp, [20, 5 * EMB], GRAPH_DT, "wrels")
        dma(wrels_t[:], wrels)
        dma(wrel_t[:, 0:10 * EMB], wrel[:, 0:10 * EMB])
        w1c_t = [T(pst, [128, 6400], CONV_DT, "wconv", bufs=4)
                 for _ in range(4)]
        dma(w1c_t[0][:], w1[:, 0:6400])
        dma(wrel_t[:, 10 * EMB:20 * EMB], wrel[:, 10 * EMB:20 * EMB])
        dma(w1c_t[1][:], w1[:, 6400:12800])

        NODE_GROUPS = [(0, E), (E, E * M), (E + E * M, L)]
        nodes_e = T(pp, [E, 532], GRAPH_DT, "nodes_e")
        nodes_m = T(pp, [E * M, 532], GRAPH_DT, "nodes_m")
        nodes_l = T(pp, [L, 532], GRAPH_DT, "nodes_l")
        node_tiles = [nodes_e, nodes_m, nodes_l]
        for gi, (goff, gsz) in enumerate(NODE_GROUPS):
            nc.vector.tensor_copy(node_tiles[gi][:, 512:532],
                                  bf16p_t[0:gsz, TYP_OFF + gi * 20:TYP_OFF + (gi + 1) * 20])
        adjf_t = [bf16p_t[0:gsz, ADJ_OFF + gi * 110:ADJ_OFF + (gi + 1) * 110]
                  for gi, (goff, gsz) in enumerate(NODE_GROUPS)]

        btrans_bc = T(pp, [128, EMB], FP32, "btrans_bc")
        nc.gpsimd.partition_broadcast(btrans_bc[:], btb_t[0:1, 0:512])

        # preload activation tables off the critical path
        scr_t = T(pp, [1, 4], FP32, "scr")
        nc.vector.memset(scr_t[:], 1.0)
        for fn in (ACT.Exp, ACT.Ln, ACT.Relu, ACT.Tanh):
            nc.scalar.activation(scr_t[:], scr_t[:], fn)

        # chaff psum: keeps the PE pstate at max through sparse phases
        chaff_ps = T(pps, [128, 128], FP32, "ps")

        def chaff(n):
            for _ in range(n):
                nc.tensor.matmul(chaff_ps[:], identp_t, identp_t,
                                 start=True, stop=True)

        chaff(24)

        # conv pad buffers; memset early, off the critical path
        xpad_t = [T(pp, [128, BIN * PW], CONV_DT, f"xpad{mt}") for mt in range(4)]
        pad1_t = [T(pp, [128, B1R * PW], CONV_DT, f"pad1_{mt}") for mt in range(2)]
        pad2_t = [T(pp, [128, B2R * PW], CONV_DT, f"pad2_{mt}") for mt in range(2)]
        for t in xpad_t + pad1_t + pad2_t:
            nc.vector.memset(t[:], 0.0)

        # ---- S2: mention embeddings + entity logsumexp nodes ----
        ps_memb = T(pps, [E * M, EMB], FP32, "ps")
        for kt in range(KT_H):
            nc.tensor.matmul(ps_memb[:], xg_t[:, kt * 66:(kt + 1) * 66],
                             wtrans_t[:, kt * EMB:(kt + 1) * EMB],
                             start=(kt == 0), stop=(kt == KT_H - 1))
        memb_t = T(pp, [E * M, EMB], FP32, "memb")
        nc.vector.tensor_add(memb_t[:], ps_memb[:], btrans_bc[0:E * M, :])
        nc.vector.tensor_copy(nodes_m[:, 0:EMB], memb_t[:])
        ememb_t = T(pp, [E * M, EMB], FP32, "ememb")
        nc.scalar.activation(ememb_t[:], memb_t[:], ACT.Exp)
        ps_ent = T(pps, [E, EMB], FP32, "ps")
        nc.tensor.matmul(ps_ent[:], g3_t, ememb_t[:], start=True, stop=True)
        nc.scalar.activation(nodes_e[:, 0:EMB], ps_ent[:], ACT.Ln)
        chaff(6)

        # ---- S3: link nodes (scale folded into gspan, not xspan) ----
        gsc_t = []
        for i in range(SPAN_T):
            a = T(pst, [128, 1], FP32, "aT", bufs=4)
            nc.vector.tensor_reduce(a[:], attl_t[:, i * 192:(i + 1) * 192],
                                    mybir.AxisListType.X, mybir.AluOpType.add)
            g = T(pp, [128, L], SEQ_DT, f"gsc{i}")
            nc.vector.tensor_scalar_mul(g[:], gspan_t[:, i * L:(i + 1) * L], a[:])
            gsc_t.append(g)
        ps_as = T(pps, [L, 1], FP32, "ps")
        for i in range(SPAN_T):
            nc.tensor.matmul(ps_as[:], gsc_t[i][:], onesb_t,
                             start=(i == 0), stop=(i == SPAN_T - 1))
        asum_t = T(pp, [L, 1], FP32, "asum")
        nc.vector.tensor_scalar_mul(asum_t[:], ps_as[:], 1.0 / (NH * LS))
        ps_lct = [T(pps, [128, L], FP32, "ps") for _ in range(KT_H)]
        for i in range(SPAN_T):
            for mt in range(KT_H):
                nc.tensor.matmul(ps_lct[mt][:],
                                 xspan_t[:, i * H + mt * 128:i * H + (mt + 1) * 128],
                                 gsc_t[i][:],
                                 start=(i == 0), stop=(i == SPAN_T - 1))
        lct_t = []
        for mt in range(KT_H):
            t = T(pp, [128, L], SEQ_DT, f"lct{mt}")
            nc.vector.tensor_scalar_mul(t[:], ps_lct[mt][:], 1.0 / (NH * LS))
            lct_t.append(t)
        bterm_t = T(pp, [L, EMB], FP32, "bterm")
        nc.vector.tensor_scalar_mul(bterm_t[:], btrans_bc[0:L, :], asum_t[:])
        ps_link = T(pps, [L, EMB], FP32, "ps")
        for kt in range(KT_H):
            nc.tensor.matmul(ps_link[:], lct_t[kt][:],
                             wtrans_t[:, kt * EMB:(kt + 1) * EMB],
                             start=(kt == 0), stop=(kt == KT_H - 1))
        nc.vector.tensor_add(nodes_l[:, 0:EMB], ps_link[:], bterm_t[:])
        chaff(6)

        # rsum[e] = sum_rows gmat[row,e] * rowsum(attm[row,:]) — first half
        arsum_t = T(pp, [128, ATTM_T], FP32, "arsum")
        for i in range(4):
            nc.vector.tensor_reduce(arsum_t[:, i:i + 1], attm_t[:, i * C:(i + 1) * C],
                                    mybir.AxisListType.X, mybir.AluOpType.add)
        # adjacency row-normalize (entity destination columns only)
        ps_rsE = T(pps, [1, 5 * E], FP32, "ps")
        for gi, (goff, gsz) in enumerate(NODE_GROUPS):
            nc.tensor.matmul(ps_rsE[:], onesb_t[0:gsz, 0:1], adjf_t[gi],
                             start=(gi == 0), stop=(gi == 2))
        rs_t = T(pp, [1, 5 * E], FP32, "rs")
        nc.vector.tensor_scalar_add(rs_t[:], ps_rsE[:], 1e-5)
        rcp_t = T(pp, [1, 5 * E], FP32, "rcp")
        nc.vector.reciprocal(rcp_t[:], rs_t[:])
        rsbc_t = T(pp, [128, 5 * E], FP32, "rsbc")
        nc.gpsimd.partition_broadcast(rsbc_t[:], rcp_t[:])
        nc.gpsimd.dma_start(w1c_t[2][:], w1[:, 12800:19200])
        rsbcb_t = T(pp, [128, 5 * E], GRAPH_DT, "rsbcb")
        nc.vector.tensor_copy(rsbcb_t[:], rsbc_t[:])
        adjn_t = []
        for gi, (goff, gsz) in enumerate(NODE_GROUPS):
            t = T(pp, [gsz, 5 * E], GRAPH_DT, f"adjn{gi}")
            nc.vector.tensor_mul(t[:], adjf_t[gi], rsbcb_t[0:gsz, :])
            adjn_t.append(t)

        msgE_t = []
        for i in range(5):
            ksz = 128 if i < 4 else 20
            ps = T(pps, [ksz, 5 * E], FP32, "ps")
            for gi, (goff, gsz) in enumerate(NODE_GROUPS):
                nc.tensor.matmul(ps[:], node_tiles[gi][:, i * 128:i * 128 + ksz],
                                 adjn_t[gi][:],
                                 start=(gi == 0), stop=(gi == 2))
            t = T(pp, [ksz, 5 * E], GRAPH_DT, f"msgE{i}")
            nc.vector.tensor_copy(t[:], ps[:])
            msgE_t.append(t)

        # ---- S4: eaT directly transposed; e_ctx = (ean@X)@W + rn*b ----
        for i in range(4, ATTM_T):
            nc.vector.tensor_reduce(arsum_t[:, i:i + 1], attm_t[:, i * C:(i + 1) * C],
                                    mybir.AxisListType.X, mybir.AluOpType.add)
        arsumb_t = T(pp, [128, ATTM_T], GRAPH_DT, "arsumb")
        nc.vector.tensor_copy(arsumb_t[:], arsum_t[:])
        ps_ct = [T(pps, [128, E], FP32, "ps") for _ in range(8)]
        for i in range(ATTM_T):
            for ct in range(8):
                nc.tensor.matmul(ps_ct[ct][:],
                                 attm_t[:, i * C + ct * 128:i * C + (ct + 1) * 128],
                                 gmat_t[:, i * E:(i + 1) * E],
                                 start=(i == 0), stop=(i == ATTM_T - 1))
        eaTb_t = []
        for ct in range(8):
            t = T(pp, [128, E], GRAPH_DT, f"eaTb{ct}")
            nc.vector.tensor_copy(t[:], ps_ct[ct][:])
            eaTb_t.append(t)
        ps_rs = T(pps, [E, 1], FP32, "ps")
        for i in range(ATTM_T):
            nc.tensor.matmul(ps_rs[:], gmat_t[:, i * E:(i + 1) * E],
                             arsumb_t[:, i:i + 1],
                             start=(i == 0), stop=(i == ATTM_T - 1))
        rsum_t = T(pp, [E, 1], FP32, "rsum")
        nc.vector.tensor_scalar_add(rsum_t[:], ps_rs[:], 1e-5)
        recip_t = T(pp, [E, 1], FP32, "recip")
        nc.vector.reciprocal(recip_t[:], rsum_t[:])
        # rn = rsum/(rsum+eps) = 1 - eps*recip
        # recip as a row vector broadcast across partitions
        ps_rt = T(pps, [1, E], FP32, "ps")
        nc.tensor.transpose(ps_rt[:], recip_t[:], f32p_t[0:E, 22:44])
        recipT_t = T(pp, [1, E], FP32, "recipT")
        nc.vector.tensor_copy(recipT_t[:], ps_rt[:])
        recipT_bc = T(pp, [128, E], FP32, "recipT_bc")
        nc.gpsimd.partition_broadcast(recipT_bc[:], recipT_t[:])

        # eanXT[h,e] tiles directly (no transposes), normalized per entity
        ps_xT = [T(pps, [128, E], FP32, "ps") for _ in range(KT_H)]
        for ct in range(8):
            for ht in range(KT_H):
                nc.tensor.matmul(ps_xT[ht][:],
                                 xfull_t[:, ct * H + ht * 128:ct * H + (ht + 1) * 128],
                                 eaTb_t[ct][:], start=(ct == 0), stop=(ct == 7))
        eanXT_t = []
        for ht in range(KT_H):
            t = T(pp, [128, E], SEQ_DT, f"eanXT{ht}")
            nc.vector.tensor_mul(t[:], ps_xT[ht][:], recipT_bc[:])
            eanXT_t.append(t)
        chaff(6)
        ps_ectxT = [T(pps, [128, E], FP32, "ps") for _ in range(4)]
        for ht in range(KT_H):
            for mt in range(4):
                nc.tensor.matmul(ps_ectxT[mt][:],
                                 wtrans_t[:, ht * EMB + mt * 128:ht * EMB + (mt + 1) * 128],
                                 eanXT_t[ht][:], start=(ht == 0), stop=(ht == KT_H - 1))
        # rnT = rowsum(ean) as a row = 1 - eps*recipT
        rnT_t = T(pp, [1, E], FP32, "rnT")
        nc.vector.tensor_scalar_mul(rnT_t[:], recipT_t[:], -1e-5)
        nc.vector.tensor_scalar_add(rnT_t[:], rnT_t[:], 1.0)
        rnT_bc = T(pp, [128, E], FP32, "rnT_bc")
        nc.gpsimd.partition_broadcast(rnT_bc[:], rnT_t[:])
        ectxT_t = []
        for mt in range(4):
            bt_ = T(pst, [128, E], FP32, "ebias", bufs=2)
            nc.vector.tensor_scalar_mul(bt_[:], rnT_bc[:], f32p_t[:, 70 + mt:71 + mt])
            t = T(pp, [128, E], PAIR_DT, f"ectxT{mt}")
            nc.vector.tensor_add(t[:], ps_ectxT[mt][:], bt_[:])
            ectxT_t.append(t)
        ectxb_t = T(pp, [E, EMB], PAIR_DT, "ectxb")
        for mt in range(4):
            ps = T(pps, [E, 64], FP32, "ps")
            psb = ps[:].bitcast(PAIR_DT)
            nc.tensor.transpose(psb, ectxT_t[mt][:], bf16p_t[:, 0:128])
            nc.vector.tensor_copy(ectxb_t[:, mt * 128:(mt + 1) * 128], psb)

        # ---- S5: RGCN (entity rows only) ----
        ps_gcnT = [T(pps, [128, E], FP32, "ps") for _ in range(4)]
        term = 0
        for r in range(5):
            for i in range(4):
                for mt in range(4):
                    nc.tensor.matmul(
                        ps_gcnT[mt][:],
                        wrel_t[:, (r * 4 + i) * EMB + mt * 128:(r * 4 + i) * EMB + (mt + 1) * 128],
                        msgE_t[i][:, r * E:(r + 1) * E],
                        start=(term == 0), stop=False)
                term += 1
        for r in range(5):
            for mt in range(4):
                nc.tensor.matmul(ps_gcnT[mt][:],
                                 wrels_t[0:20, r * EMB + mt * 128:r * EMB + (mt + 1) * 128],
                                 msgE_t[4][0:20, r * E:(r + 1) * E],
                                 start=False, stop=(r == 4))
        nc.gpsimd.dma_start(w1c_t[3][:], w1[:, 19200:25600])
        gcnT_t = []
        for mt in range(4):
            t = T(pp, [128, E], PAIR_DT, f"gcnT{mt}")
            nc.scalar.activation(t[:], ps_gcnT[mt][:], ACT.Relu,
                                 bias=f32p_t[:, 66 + mt:67 + mt])
            gcnT_t.append(t)
        entb_t = T(pp, [E, EMB], PAIR_DT, "entb")
        for mt in range(4):
            ps = T(pps, [E, 64], FP32, "ps")
            psb = ps[:].bitcast(PAIR_DT)
            nc.tensor.transpose(psb, gcnT_t[mt][:], bf16p_t[:, 0:128])
            nc.vector.tensor_copy(entb_t[:, mt * 128:(mt + 1) * 128], psb)
        featT = [None] * 16
        for srх in range(2):
            for mt in range(4):
                ps = T(pps, [128, PH2], FP32, "ps")
                nc.tensor.matmul(ps[:], entb_t[:, mt * 128:(mt + 1) * 128],
                                 shst_t[:, srх * PH2:(srх + 1) * PH2],
                                 start=True, stop=True)
                t = T(pp, [128, PH2], PAIR_DT, f"featT{4 * srх + mt}")
                nc.vector.tensor_copy(t[:], ps[:])
                featT[4 * srх + mt] = t
        for mt in range(4):
            t = T(pp, [128, PH2], PAIR_DT, f"featT{12 + mt}")
            nc.vector.tensor_mul(t[:], featT[mt][:], featT[4 + mt][:])
            featT[12 + mt] = t

        # ---- S6: relation map x (bf16 transposes + row selection) ----
        ps_c1 = [T(pps, [128, R1 * S], FP32, "ps") for _ in range(2)]
        entT_t, ectxTv_t = gcnT_t, ectxT_t
        entS_t, ectxS_t = [], []
        for mt in range(4):
            for src_t, dst_list, nf in ((entb_t, entS_t, "entS"),
                                        (ectxb_t, ectxS_t, "ectxS")):
                ps = T(pps, [128, 17], FP32, "ps")
                nc.tensor.matmul(ps[:], src_t[:, mt * 128:(mt + 1) * 128], psel_t,
                                 start=True, stop=True)
                t = T(pp, [128, 17], PAIR_DT, f"{nf}{mt}")
                nc.vector.tensor_copy(t[:], ps[:])
                dst_list.append(t)
            t1 = T(pst, [128, 17 * S], FP32, "xtmp", bufs=2)
            nc.vector.tensor_mul(
                t1[:].rearrange("p (a b) -> p a b", a=17, b=S),
                entS_t[mt][:].unsqueeze(2).to_broadcast((128, 17, S)),
                entT_t[mt][:].unsqueeze(1).to_broadcast((128, 17, S)))
            t2 = T(pst, [128, 17 * S], FP32, "xtmp", bufs=2)
            nc.vector.tensor_mul(
                t2[:].rearrange("p (a b) -> p a b", a=17, b=S),
                ectxS_t[mt][:].unsqueeze(2).to_broadcast((128, 17, S)),
                ectxTv_t[mt][:].unsqueeze(1).to_broadcast((128, 17, S)))
            inner = xpad_t[mt][:].rearrange("p (a b) -> p a b", a=BIN, b=PW)[
                :, 2:2 + 17, 2:2 + S]
            nc.vector.tensor_add(inner,
                                 t1[:].rearrange("p (a b) -> p a b", a=17, b=S),
                                 t2[:].rearrange("p (a b) -> p a b", a=17, b=S))
            if mt == 0:
                chaff(16)

        # conv1: 512 -> 256
        for kt in range(4):
            w = w1c_t[kt]
            for tap in range(25):
                di, dj = divmod(tap, 5)
                rhs = xpad_t[kt][:].rearrange("p (a b) -> p a b", a=BIN, b=PW)[
                    :, di:di + R1, dj:dj + S]
                for mt in range(2):
                    nc.tensor.matmul(ps_c1[mt][:],
                                     w[:, tap * 256 + mt * 128:tap * 256 + (mt + 1) * 128],
                                     rhs, start=(kt == 0 and tap == 0),
                                     stop=(kt == 3 and tap == 24))
        for mt in range(2):
            inner = pad1_t[mt][:].rearrange("p (a b) -> p a b", a=B1R, b=PW)[
                :, 2:2 + R1, 2:2 + S]
            nc.scalar.activation(inner,
                                 ps_c1[mt][:].rearrange("p (a b) -> p a b", a=R1, b=S),
                                 ACT.Relu, bias=biasp_t[:, mt:mt + 1])

        # conv2: 256 -> 256
        ps_c2 = [T(pps, [128, R2 * S], FP32, "ps") for _ in range(2)]
        for kt in range(2):
            w = T(pst, [128, 6400], CONV_DT, "wconv", bufs=4)
            dma(w[:], w2[:, kt * 6400:(kt + 1) * 6400])
            for tap in range(25):
                di, dj = divmod(tap, 5)
                rhs = pad1_t[kt][:].rearrange("p (a b) -> p a b", a=B1R, b=PW)[
                    :, di:di + R2, dj:dj + S]
                for mt in range(2):
                    nc.tensor.matmul(ps_c2[mt][:],
                                     w[:, tap * 256 + mt * 128:tap * 256 + (mt + 1) * 128],
                                     rhs, start=(kt == 0 and tap == 0),
                                     stop=(kt == 1 and tap == 24))
        for mt in range(2):
            inner = pad2_t[mt][:].rearrange("p (a b) -> p a b", a=B2R, b=PW)[
                :, 2:2 + R2, 2:2 + S]
            nc.scalar.activation(inner,
                                 ps_c2[mt][:].rearrange("p (a b) -> p a b", a=R2, b=S),
                                 ACT.Relu, bias=biasp_t[:, 2 + mt:3 + mt])

        # wht streamed now so its 4.2MB overlaps conv2/conv3 compute
        whtc = []
        for c in range(2):
            t = T(pst, [128, 8 * 1024], PAIR_DT, "bigreuse", bufs=2)
            dma(t[:], wht[:, c * 8192:(c + 1) * 8192])
            whtc.append(t)

        # retire chaff psum so it isn't dead code
        warm_sb = T(pp, [128, 128], FP32, "warm_sb")
        nc.vector.tensor_copy(warm_sb[:], chaff_ps[:])
        warm_dram = pdram.tile([128, 128], FP32, name="warm_dram")
        dma(warm_dram[:], warm_sb[:])

        # conv3: 256 -> 512, four (kt, taphalf) chunks
        x3_t = [T(pp, [128, SP2], PAIR_DT, f"x3_{mt}") for mt in range(4)]
        ps_c3 = [T(pps, [128, SP2], FP32, "ps") for _ in range(4)]
        for kt in range(2):
            for taps in (range(0, 13), range(13, 25)):
                w = T(pst, [128, len(taps) * EMB], CONV_DT, "wconv", bufs=4)
                dma(w[:], w3[:, (kt * 25 + taps.start) * EMB:(kt * 25 + taps.stop) * EMB])
                for tj, tap in enumerate(taps):
                    di, dj = divmod(tap, 5)
                    rhs = pad2_t[kt][:].rearrange("p (a b) -> p a b", a=B2R, b=PW)[
                        :, di:di + R3, dj:dj + S]
                    for mt in range(4):
                        nc.tensor.matmul(ps_c3[mt][:],
                                         w[:, tj * EMB + mt * 128:tj * EMB + (mt + 1) * 128],
                                         rhs, start=(kt == 0 and tap == 0),
                                         stop=(kt == 1 and tap == 24))
        for mt in range(4):
            nc.scalar.activation(x3_t[mt][:], ps_c3[mt][:], ACT.Relu,
                                 bias=biasp_t[:, 4 + mt:5 + mt])

        # ---- S7: pair features + classifier ----
        SP_TILES = [(0, 128), (128, SP2 - 128)]
        x3T_t = [T(pp, [sz, EMB], PAIR_DT, f"x3T{i}")
                 for i, (off, sz) in enumerate(SP_TILES)]
        for i, (off, sz) in enumerate(SP_TILES):
            for src in range(4):
                ps = T(pps, [sz, 64], FP32, "ps")
                psb = ps[:].bitcast(PAIR_DT)
                nc.tensor.transpose(psb, x3_t[src][:, off:off + sz], bf16p_t[:, 0:128])
                nc.vector.tensor_copy(x3T_t[i][:, src * 128:(src + 1) * 128], psb)
        chaff(20)

        smp_t = T(pp, [128, 2 * PH2], PAIR_DT, "smp")
        dma(smp_t[:], smp)

        for mt in range(4):
            ps = T(pps, [128, PH2], FP32, "ps")
            for i, (off, sz) in enumerate(SP_TILES):
                nc.tensor.matmul(ps[:], x3T_t[i][0:sz, mt * 128:(mt + 1) * 128],
                                 smp_t[0:sz, i * PH2:(i + 1) * PH2],
                                 start=(i == 0), stop=(i == 1))
            t = T(pp, [128, PH2], PAIR_DT, f"featT{8 + mt}")
            nc.vector.tensor_copy(t[:], ps[:])
            featT[8 + mt] = t

        chaff(12)
        ps_ht = [T(pps, [128, PH2], FP32, "ps") for _ in range(8)]
        for kt in range(16):
            wv = whtc[kt // 8]
            for mt in range(8):
                nc.tensor.matmul(
                    ps_ht[mt][:],
                    wv[:, (kt % 8) * 1024 + mt * 128:(kt % 8) * 1024 + (mt + 1) * 128],
                    featT[kt][:], start=(kt == 0), stop=(kt == 15))
        htT_t = []
        for mt in range(8):
            t = T(pp, [128, PH2], PAIR_DT, f"htT{mt}")
            nc.scalar.activation(t[:], ps_ht[mt][:], ACT.Tanh,
                                 bias=biasp_t[:, 12 + mt:13 + mt])
            htT_t.append(t)

        chaff(6)
        wbil_t = T(pp, [128, 8 * 97], PAIR_DT, "wbil")
        dma(wbil_t[:], wbil)
        ps_out = T(pps, [97, PH2], FP32, "ps")
        for kt in range(8):
            nc.tensor.matmul(ps_out[:], wbil_t[:, kt * 97:(kt + 1) * 97],
                             htT_t[kt][:], start=(kt == 0), stop=(kt == 7))
        out_t = T(pp, [97, PH2], FP32, "out")
        nc.vector.tensor_scalar_add(out_t[:], ps_out[:], biasp_t[0:97, 20:21])
        dma(outt, out_t[:])

    nc.compile()
    return nc


_PROG = None


def _get_prog():
    global _PROG
    if _PROG is None:
        _PROG = build_program()
    return _PROG


def _np(dt):
    return _NPDT[dt]


def _pack_rows(a, ntiles, dt):
    """[ntiles*128, W] (zero-padded) -> [128, ntiles*W] with tile i at
    column block i."""
    r, w = a.shape
    pad = ntiles * 128 - r
    if pad:
        a = np.concatenate([a, np.zeros((pad, w), a.dtype)], axis=0)
    return np.ascontiguousarray(
        a.reshape(ntiles, 128, w).transpose(1, 0, 2).reshape(128, ntiles * w),
        _np(dt))


def _pack_conv(w, flip, dt):
    """conv weight OIHW -> tap-major per-kt chunks [128, ...]."""
    w = np.asarray(w, np.float32)
    if flip:
        w = w[:, :, ::-1, :]
    oc, ic, _, _ = w.shape
    t = w.transpose(2, 3, 1, 0).reshape(25, ic, oc)   # (tap, ic, oc)
    nkt = ic // 128
    chunks = [np.ascontiguousarray(
        t[:, kt * 128:(kt + 1) * 128, :].transpose(1, 0, 2).reshape(128, 25 * oc))
        for kt in range(nkt)]
    return np.ascontiguousarray(np.concatenate(chunks, axis=1), _np(dt))


def _pack_conv3(w, flip, dt):
    """conv3 weights as (kt, taphalf) chunks: [128, 25*512] per kt with
    taps in order — column block (kt*25 + tap)*512."""
    w = np.asarray(w, np.float32)
    if flip:
        w = w[:, :, ::-1, :]
    t = w.transpose(2, 3, 1, 0).reshape(25, IC, EMB)
    chunks = [np.ascontiguousarray(
        t[:, kt * 128:(kt + 1) * 128, :].transpose(1, 0, 2).reshape(128, 25 * EMB))
        for kt in range(2)]
    return np.ascontiguousarray(np.concatenate(chunks, axis=1), _np(dt))


def _shared_inputs(inputs):
    f32 = np.float32
    sh = {}
    fp = np.zeros((128, 74), f32)
    fp[:, 0] = np.asarray(inputs["conv1_b"], f32)[0:128]
    fp[:, 1] = np.asarray(inputs["conv1_b"], f32)[128:256]
    fp[:, 2] = np.asarray(inputs["conv2_b"], f32)[0:128]
    fp[:, 3] = np.asarray(inputs["conv2_b"], f32)[128:256]
    for mt in range(4):
        fp[:, 4 + mt] = np.asarray(inputs["conv3_b"], f32)[mt * 128:(mt + 1) * 128]
    for mt in range(8):
        fp[:, 12 + mt] = np.asarray(inputs["ht_b"], f32)[mt * 128:(mt + 1) * 128]
    fp[0:97, 20] = np.asarray(inputs["bil_b"], f32)
    fp[:, 21] = 1.0
    fp[0:22, 22:44] = np.eye(22, dtype=f32)
    fp[0:E * M, 44:66] = np.kron(np.eye(E, dtype=f32), np.ones((M, 1), f32))
    for mt in range(4):
        fp[:, 66 + mt] = np.asarray(inputs["b_rgcn"], f32)[mt * 128:(mt + 1) * 128]
        fp[:, 70 + mt] = np.asarray(inputs["b_trans"], f32)[mt * 128:(mt + 1) * 128]
    sh["f32p"] = fp
    bb = np.zeros((1, 1024), f32)
    bb[0, 0:512] = np.asarray(inputs["b_trans"], f32)
    bb[0, 512:1024] = np.asarray(inputs["b_rgcn"], f32)
    sh["btb"] = bb
    bt = np.zeros((128, 1206), np.float32)
    bt[:, 0:128] = np.eye(128, dtype=f32)
    bt[:, 524:644] = _pack_rows(np.kron(np.eye(L, dtype=f32),
                                        np.ones((LS, 1), f32)), SPAN_T, FP32)
    bt[:, 644:798] = _pack_rows(np.kron(np.eye(E, dtype=f32),
                                        np.ones((M * NH, 1), f32) / (M * NH)),
                                ATTM_T, FP32)
    bt[:, 1205] = 1.0
    sh["bf16p_base"] = bt
    sh["wtrans"] = _pack_rows(np.asarray(inputs["W_trans"], f32), KT_H, SEQ_DT)
    # wrel: 5 relations (4 + self); big k-tiles [128] and the 20-row tail
    wr = np.zeros((5, 532, EMB), f32)
    wr[0:4] = np.asarray(inputs["W_rel"], f32)
    wr[4] = np.asarray(inputs["W_self"], f32)
    wrb = wr[:, 0:512, :].reshape(5, 4, 128, EMB)   # (r, i, 128, 512)
    sh["wrel"] = np.ascontiguousarray(
        wrb.transpose(2, 0, 1, 3).reshape(128, 20 * EMB), _np(GRAPH_DT))
    sh["wrels"] = np.ascontiguousarray(
        wr[:, 512:532, :].transpose(1, 0, 2).reshape(20, 5 * EMB), _np(GRAPH_DT))
    sh["w1"] = [_pack_conv(inputs["conv1_w"], fl, CONV_DT) for fl in (0, 1)]
    sh["w2"] = [_pack_conv(inputs["conv2_w"], fl, CONV_DT) for fl in (0, 1)]
    sh["w3"] = [_pack_conv3(inputs["conv3_w"], fl, CONV_DT) for fl in (0, 1)]
    sh["wht"] = _pack_rows(np.asarray(inputs["ht_W"], f32), 16, PAIR_DT)
    sh["wbil"] = _pack_rows(np.asarray(inputs["bil_W"], f32), 8, PAIR_DT)
    psel = []
    for fl in (0, 1):
        pm = np.zeros((E, 17), f32)
        for r in range(17):
            pm[(21 - r) if fl else r, r] = 1.0
        psel.append(pm)
    sh["psel"] = psel
    return sh


def _pair_idx(hts_b, hh):
    h = np.asarray(hts_b)[:, 0]
    mask = (h <= 10) if hh == 0 else (h >= 11)
    idx = np.nonzero(mask)[0]
    if len(idx) > PH2:
        raise RuntimeError(f"pair overflow: {len(idx)} > {PH2}")
    return idx


def _core_inputs(inputs, shared, b, hh):
    f32 = np.float32
    X = np.asarray(inputs["sequence_output"][b], f32)
    att = np.asarray(inputs["attention"][b], f32)
    adj = np.asarray(inputs["adjacency"][b], f32)
    mf = np.asarray(inputs["mention_idx"][b]).reshape(-1).astype(np.int64)
    ls = np.asarray(inputs["link_start"][b]).reshape(-1).astype(np.int64)
    ntypes = np.asarray(inputs["node_types"][b]).astype(np.int64)
    hts = np.asarray(inputs["hts"][b]).astype(np.int64)

    m = {k: shared[k] for k in
         ("f32p", "btb", "wtrans", "wrel", "wrels", "wht", "wbil")}
    m["w1"] = shared["w1"][hh]
    m["w2"] = shared["w2"][hh]
    m["w3"] = shared["w3"][hh]
    bt = shared["bf16p_base"].copy()
    bt[:, 128:524] = _pack_rows(np.ascontiguousarray(X[mf].T), KT_H, FP32)
    bt[0:E, 798:815] = shared["psel"][hh]
    te = np.asarray(inputs["type_embed"], f32)[ntypes]
    adjc = np.concatenate([adj[r].T[:, 0:E] for r in range(4)]
                          + [np.eye(NN, E, dtype=f32)], axis=1)
    for gi, (goff, gsz) in enumerate([(0, E), (E, E * M), (E + E * M, L)]):
        bt[0:gsz, 815 + gi * 20:835 + gi * 20] = te[goff:goff + gsz]
        bt[0:gsz, 875 + gi * 110:985 + gi * 110] = adjc[goff:goff + gsz]
    m["bf16p"] = np.ascontiguousarray(bt, _np(PAIR_DT))
    pos = ls[:, None] + np.arange(LS)
    m["xspan"] = _pack_rows(X[pos.reshape(-1)], SPAN_T, SEQ_DT)
    m["xfull"] = _pack_rows(X, 8, SEQ_DT)
    rows = att[:, mf, :]
    m["attm"] = _pack_rows(rows.transpose(1, 0, 2).reshape(E * M * NH, C),
                           ATTM_T, GRAPH_DT)
    attl = np.empty((L * LS, NH * LS), f32)
    for l in range(L):
        blk = att[:, pos[l], :][:, :, pos[l]]
        attl[l * LS:(l + 1) * LS, :] = blk.transpose(2, 0, 1).reshape(LS, NH * LS)
    m["attl"] = _pack_rows(attl, SPAN_T, GRAPH_DT)

    idx = _pair_idx(hts, hh)
    pr = hts[idx]
    n = len(idx)
    shm = np.zeros((E, 2 * PH2), f32)
    shm[pr[:, 0], np.arange(n)] = 1.0
    shm[pr[:, 1], PH2 + np.arange(n)] = 1.0
    m["shst"] = np.ascontiguousarray(shm, _np(PAIR_DT))
    loc_r = pr[:, 0] if hh == 0 else (21 - pr[:, 0])
    smm = np.zeros((SP2, PH2), f32)
    smm[loc_r * S + pr[:, 1], np.arange(n)] = 1.0
    m["smp"] = _pack_rows(smm, 2, PAIR_DT)
    return m


def kernel(**inputs):
    nc = _get_prog()
    shared = _shared_inputs(inputs)
    in_maps = []
    for b in range(B):
        for hh in range(2):
            in_maps.append(_core_inputs(inputs, shared, b, hh))
    res = run_bass_kernel_spmd(nc, in_maps, core_ids=list(range(8)))
    out = np.empty((B, P, 97), np.float32)
    for b in range(B):
        for hh in range(2):
            idx = _pair_idx(np.asarray(inputs["hts"][b]), hh)
            r = np.asarray(res.results[2 * b + hh]["outt"], np.float32)
            out[b, idx, :] = r[:, 0:len(idx)].T
    return out


# revision 34
# speedup vs baseline: 1.0300x; 1.0024x over previous
"""Trainium2 Bass kernel for nn_DocREModel (8-core SPMD).

Sharding: data-parallel over the 4 documents x 2 halves = 8 cores.
Each doc's two cores duplicate the (cheap) graph phase, then split the
expensive conv reasoning stack SPATIALLY: core A computes output rows
0..10 of the 22x22 relation map, core B rows 11..21.  B works on a
row-FLIPPED frame with row-flipped conv taps so both cores run the
identical instruction stream (SPMD) -- only the data differs.  Pairs
are split by head entity (h<=10 vs h>=11), padded to PH2=288 columns.

e_ctx is reassociated:  ean @ (X @ W + b) == (ean @ X) @ W + rowsum(ean)*b,
which removes the full [1024,512] seq transform.

All floating-point arithmetic runs on device.  Host does only
index-driven data movement: batch slicing, transposes, row gathers at
integer indices, one-hot/selector construction, and layout packing.

DMA discipline: every sizable tensor is packed host-side as [128, W]
with per-partition-contiguous rows so each tensor is ONE dma_start
(the SP sequencer pays ~565ns per dma_start; the baseline's 334 DMAs
serialized 205us of sync time).
"""

import numpy as np
from contextlib import ExitStack

import concourse.bass as bass
import concourse.bacc as bacc
import concourse.tile as tile
import concourse.mybir as mybir
from concourse.bass_utils import run_bass_kernel_spmd

import ml_dtypes

FP32 = mybir.dt.float32
BF16 = mybir.dt.bfloat16

SEQ_DT = BF16
CONV_DT = BF16
PAIR_DT = BF16
GRAPH_DT = BF16

_NPDT = {FP32: np.float32, BF16: ml_dtypes.bfloat16}

B, C, H, NH = 4, 1024, 768, 12
E, M, L, LS = 22, 3, 30, 16
NN, EMB = 118, 512
P = 462
IC = 256
S = 22              # spatial side of relation map
PW = 26             # padded column width (2+22+2)
R1, R2, R3 = 15, 13, 11   # conv1/2/3 computed rows (half + halo)
BIN, B1R, B2R = 21, 19, 17  # padded row counts of conv input/1/2 buffers
SP2 = R3 * S        # 242 flattened conv3 output
PH2 = 288           # padded pairs per core (h-split of 462)
ACT = mybir.ActivationFunctionType
KT_H = H // 128     # 6
ATTM_T = 7          # ceil(792/128)
SPAN_T = 4          # ceil(480/128)


def build_program():
    nc = bacc.Bacc("TRN2", target_bir_lowering=False, debug=False)

    dins = {}

    def din(name, shape, dt=FP32):
        dins[name] = nc.dram_tensor(name, shape, dt, kind="ExternalInput").ap()
        return dins[name]

    f32p = din("f32p", [128, 74], FP32)       # biasp|ident22|g3|brgcnT|btransT
    btb = din("btb", [1, 1024], FP32)         # b_trans|b_rgcn
    bf16p = din("bf16p", [128, 1206], PAIR_DT)  # identp|xg|gspan|gmat|psel|typ|adjf|ones
    wtrans = din("wtrans", [128, KT_H * EMB], SEQ_DT)
    attl = din("attl", [128, SPAN_T * NH * LS], GRAPH_DT)
    xspan = din("xspan", [128, SPAN_T * H], SEQ_DT)
    attm = din("attm", [128, ATTM_T * C], GRAPH_DT)
    xfull = din("xfull", [128, 8 * H], SEQ_DT)
    wrel = din("wrel", [128, 20 * EMB], GRAPH_DT)  # (r,i<4)-tile at col (r*4+i)*512
    wrels = din("wrels", [20, 5 * EMB], GRAPH_DT)  # the 20-row k-tiles, per relation
    w1 = din("w1", [128, 25600], CONV_DT)     # 4 kt-chunks of [128, 25*256]
    w2 = din("w2", [128, 12800], CONV_DT)     # 2 kt-chunks
    w3 = din("w3", [128, 25600], CONV_DT)     # (kt, taphalf) chunks
    shst = din("shst", [E, 2 * PH2], PAIR_DT)
    smp = din("smp", [128, 2 * PH2], PAIR_DT)  # 2 row-tiles of sm
    wht = din("wht", [128, 16 * 1024], PAIR_DT)
    wbil = din("wbil", [128, 8 * 97], PAIR_DT)
    outt = nc.dram_tensor("outt", [97, PH2], FP32, kind="ExternalOutput").ap()

    with tile.TileContext(nc) as tc, ExitStack() as ctx:
        pp = ctx.enter_context(tc.tile_pool(name="persist", bufs=1))
        pst = ctx.enter_context(tc.tile_pool(name="stream", bufs=1))
        pps = ctx.enter_context(tc.tile_pool(name="psum", bufs=8, space="PSUM"))
        pdram = ctx.enter_context(tc.tile_pool(name="dram", bufs=1, space="DRAM"))

        dma = nc.sync.dma_start

        def T(pool, shape, dt, tag, bufs=None):
            return pool.tile(shape, dt, tag=tag, name=tag, bufs=bufs)

        # ---- persistent tiles; two packed small-tensor DMAs + big tensors ----
        bf16p_t = T(pp, [128, 1206], PAIR_DT, "bf16p")
        dma(bf16p_t[:], bf16p)
        f32p_t = T(pp, [128, 74], FP32, "f32p")
        dma(f32p_t[:], f32p)
        btb_t = T(pp, [1, 1024], FP32, "btb")
        dma(btb_t[:], btb)
        identp_t = bf16p_t[:, 0:128]
        xg_t = bf16p_t[:, 128:524]
        gspan_t = bf16p_t[:, 524:644]
        gmat_t = bf16p_t[:, 644:798]
        psel_t = bf16p_t[0:E, 798:815]
        TYP_OFF, ADJ_OFF = 815, 875
        onesb_t = bf16p_t[:, 1205:1206]
        biasp_t = f32p_t[:, 0:22]
        g3_t = f32p_t[0:E * M, 44:66]
        wtrans_t = T(pp, [128, KT_H * EMB], SEQ_DT, "wtrans")
        dma(wtrans_t[:], wtrans)
        attl_t = T(pp, [128, SPAN_T * NH * LS], GRAPH_DT, "attl")
        dma(attl_t[:], attl)
        attm_t = T(pst, [128, ATTM_T * C], GRAPH_DT, "bigreuse", bufs=2)
        dma(attm_t[:, 0:4 * C], attm[:, 0:4 * C])
        xspan_t = T(pp, [128, SPAN_T * H], SEQ_DT, "xspan")
        dma(xspan_t[:], xspan)
        shst_t = T(pp, [E, 2 * PH2], PAIR_DT, "shst")
        dma(shst_t[:], shst)
        dma(attm_t[:, 4 * C:ATTM_T * C], attm[:, 4 * C:ATTM_T * C])
        xfull_t = T(pst, [128, 8 * H], SEQ_DT, "bigreuse", bufs=2)
        dma(xfull_t[:, 0:4 * H], xfull[:, 0:4 * H])
        dma(xfull_t[:, 4 * H:8 * H], xfull[:, 4 * H:8 * H])
        wrel_t = T(pp, [128, 20 * EMB], GRAPH_DT, "wrel")
        wrels_t = T(pp, [20, 5 * EMB], GRAPH_DT, "wrels")
        dma(wrels_t[:], wrels)
        dma(wrel_t[:, 0:10 * EMB], wrel[:, 0:10 * EMB])
        w1c_t = [T(pst, [128, 6400], CONV_DT, "wconv", bufs=5)
                 for _ in range(4)]
        dma(w1c_t[0][:], w1[:, 0:6400])
        dma(wrel_t[:, 10 * EMB:20 * EMB], wrel[:, 10 * EMB:20 * EMB])
        dma(w1c_t[1][:, 0:3200], w1[:, 6400:9600])
        dma(w1c_t[1][:, 3200:6400], w1[:, 9600:12800])

        NODE_GROUPS = [(0, E), (E, E * M), (E + E * M, L)]
        nodes_e = T(pp, [E, 532], GRAPH_DT, "nodes_e")
        nodes_m = T(pp, [E * M, 532], GRAPH_DT, "nodes_m")
        nodes_l = T(pp, [L, 532], GRAPH_DT, "nodes_l")
        node_tiles = [nodes_e, nodes_m, nodes_l]
        for gi, (goff, gsz) in enumerate(NODE_GROUPS):
            nc.vector.tensor_copy(node_tiles[gi][:, 512:532],
                                  bf16p_t[0:gsz, TYP_OFF + gi * 20:TYP_OFF + (gi + 1) * 20])
        adjf_t = [bf16p_t[0:gsz, ADJ_OFF + gi * 110:ADJ_OFF + (gi + 1) * 110]
                  for gi, (goff, gsz) in enumerate(NODE_GROUPS)]

        btrans_bc = T(pp, [128, EMB], FP32, "btrans_bc")
        nc.gpsimd.partition_broadcast(btrans_bc[:], btb_t[0:1, 0:512])

        # preload activation tables off the critical path
        scr_t = T(pp, [1, 4], FP32, "scr")
        nc.vector.memset(scr_t[:], 1.0)
        for fn in (ACT.Exp, ACT.Ln, ACT.Relu, ACT.Tanh):
            nc.scalar.activation(scr_t[:], scr_t[:], fn)

        # chaff psum: keeps the PE pstate at max through sparse phases
        chaff_ps = T(pps, [128, 128], FP32, "ps")

        def chaff(n):
            for _ in range(n):
                nc.tensor.matmul(chaff_ps[:], identp_t, identp_t,
                                 start=True, stop=True)

        chaff(24)

        # conv pad buffers; memset early, off the critical path
        xpad_t = [T(pp, [128, BIN * PW], CONV_DT, f"xpad{mt}") for mt in range(4)]
        pad1_t = [T(pp, [128, B1R * PW], CONV_DT, f"pad1_{mt}") for mt in range(2)]
        pad2_t = [T(pp, [128, B2R * PW], CONV_DT, f"pad2_{mt}") for mt in range(2)]
        for t in xpad_t + pad1_t + pad2_t:
            nc.vector.memset(t[:], 0.0)

        # ---- S2: mention embeddings + entity logsumexp nodes ----
        ps_memb = T(pps, [E * M, EMB], FP32, "ps")
        for kt in range(KT_H):
            nc.tensor.matmul(ps_memb[:], xg_t[:, kt * 66:(kt + 1) * 66],
                             wtrans_t[:, kt * EMB:(kt + 1) * EMB],
                             start=(kt == 0), stop=(kt == KT_H - 1))
        memb_t = T(pp, [E * M, EMB], FP32, "memb")
        nc.vector.tensor_add(memb_t[:], ps_memb[:], btrans_bc[0:E * M, :])
        nc.vector.tensor_copy(nodes_m[:, 0:EMB], memb_t[:])
        ememb_t = T(pp, [E * M, EMB], FP32, "ememb")
        nc.scalar.activation(ememb_t[:], memb_t[:], ACT.Exp)
        ps_ent = T(pps, [E, EMB], FP32, "ps")
        nc.tensor.matmul(ps_ent[:], g3_t, ememb_t[:], start=True, stop=True)
        nc.scalar.activation(nodes_e[:, 0:EMB], ps_ent[:], ACT.Ln)
        chaff(6)

        # ---- S3: link nodes (scale folded into gspan, not xspan) ----
        gsc_t = []
        for i in range(SPAN_T):
            a = T(pst, [128, 1], FP32, "aT", bufs=4)
            nc.vector.tensor_reduce(a[:], attl_t[:, i * 192:(i + 1) * 192],
                                    mybir.AxisListType.X, mybir.AluOpType.add)
            g = T(pp, [128, L], SEQ_DT, f"gsc{i}")
            nc.vector.tensor_scalar_mul(g[:], gspan_t[:, i * L:(i + 1) * L], a[:])
            gsc_t.append(g)
        ps_as = T(pps, [L, 1], FP32, "ps")
        for i in range(SPAN_T):
            nc.tensor.matmul(ps_as[:], gsc_t[i][:], onesb_t,
                             start=(i == 0), stop=(i == SPAN_T - 1))
        asum_t = T(pp, [L, 1], FP32, "asum")
        nc.vector.tensor_scalar_mul(asum_t[:], ps_as[:], 1.0 / (NH * LS))
        ps_lct = [T(pps, [128, L], FP32, "ps") for _ in range(KT_H)]
        for i in range(SPAN_T):
            for mt in range(KT_H):
                nc.tensor.matmul(ps_lct[mt][:],
                                 xspan_t[:, i * H + mt * 128:i * H + (mt + 1) * 128],
                                 gsc_t[i][:],
                                 start=(i == 0), stop=(i == SPAN_T - 1))
        lct_t = []
        for mt in range(KT_H):
            t = T(pp, [128, L], SEQ_DT, f"lct{mt}")
            nc.vector.tensor_scalar_mul(t[:], ps_lct[mt][:], 1.0 / (NH * LS))
            lct_t.append(t)
        bterm_t = T(pp, [L, EMB], FP32, "bterm")
        nc.vector.tensor_scalar_mul(bterm_t[:], btrans_bc[0:L, :], asum_t[:])
        ps_link = T(pps, [L, EMB], FP32, "ps")
        for kt in range(KT_H):
            nc.tensor.matmul(ps_link[:], lct_t[kt][:],
                             wtrans_t[:, kt * EMB:(kt + 1) * EMB],
                             start=(kt == 0), stop=(kt == KT_H - 1))
        nc.vector.tensor_add(nodes_l[:, 0:EMB], ps_link[:], bterm_t[:])
        chaff(6)

        # rsum[e] = sum_rows gmat[row,e] * rowsum(attm[row,:]) — first half
        arsum_t = T(pp, [128, ATTM_T], FP32, "arsum")
        for i in range(4):
            nc.vector.tensor_reduce(arsum_t[:, i:i + 1], attm_t[:, i * C:(i + 1) * C],
                                    mybir.AxisListType.X, mybir.AluOpType.add)
        # adjacency row-normalize (entity destination columns only)
        ps_rsE = T(pps, [1, 5 * E], FP32, "ps")
        for gi, (goff, gsz) in enumerate(NODE_GROUPS):
            nc.tensor.matmul(ps_rsE[:], onesb_t[0:gsz, 0:1], adjf_t[gi],
                             start=(gi == 0), stop=(gi == 2))
        rs_t = T(pp, [1, 5 * E], FP32, "rs")
        nc.vector.tensor_scalar_add(rs_t[:], ps_rsE[:], 1e-5)
        rcp_t = T(pp, [1, 5 * E], FP32, "rcp")
        nc.vector.reciprocal(rcp_t[:], rs_t[:])
        rsbc_t = T(pp, [128, 5 * E], FP32, "rsbc")
        nc.gpsimd.partition_broadcast(rsbc_t[:], rcp_t[:])
        nc.gpsimd.dma_start(w1c_t[2][:, 0:3200], w1[:, 12800:16000])
        nc.gpsimd.dma_start(w1c_t[2][:, 3200:6400], w1[:, 16000:19200])
        rsbcb_t = T(pp, [128, 5 * E], GRAPH_DT, "rsbcb")
        nc.vector.tensor_copy(rsbcb_t[:], rsbc_t[:])
        adjn_t = []
        for gi, (goff, gsz) in enumerate(NODE_GROUPS):
            t = T(pp, [gsz, 5 * E], GRAPH_DT, f"adjn{gi}")
            nc.vector.tensor_mul(t[:], adjf_t[gi], rsbcb_t[0:gsz, :])
            adjn_t.append(t)

        msgE_t = []
        for i in range(5):
            ksz = 128 if i < 4 else 20
            ps = T(pps, [ksz, 5 * E], FP32, "ps")
            for gi, (goff, gsz) in enumerate(NODE_GROUPS):
                nc.tensor.matmul(ps[:], node_tiles[gi][:, i * 128:i * 128 + ksz],
                                 adjn_t[gi][:],
                                 start=(gi == 0), stop=(gi == 2))
            t = T(pp, [ksz, 5 * E], GRAPH_DT, f"msgE{i}")
            nc.vector.tensor_copy(t[:], ps[:])
            msgE_t.append(t)

        # ---- S4: eaT directly transposed; e_ctx = (ean@X)@W + rn*b ----
        for i in range(4, ATTM_T):
            nc.vector.tensor_reduce(arsum_t[:, i:i + 1], attm_t[:, i * C:(i + 1) * C],
                                    mybir.AxisListType.X, mybir.AluOpType.add)
        arsumb_t = T(pp, [128, ATTM_T], GRAPH_DT, "arsumb")
        nc.vector.tensor_copy(arsumb_t[:], arsum_t[:])
        ps_ct = [T(pps, [128, E], FP32, "ps") for _ in range(8)]
        for i in range(ATTM_T):
            for ct in range(8):
                nc.tensor.matmul(ps_ct[ct][:],
                                 attm_t[:, i * C + ct * 128:i * C + (ct + 1) * 128],
                                 gmat_t[:, i * E:(i + 1) * E],
                                 start=(i == 0), stop=(i == ATTM_T - 1))
        eaTb_t = []
        for ct in range(8):
            t = T(pp, [128, E], GRAPH_DT, f"eaTb{ct}")
            nc.vector.tensor_copy(t[:], ps_ct[ct][:])
            eaTb_t.append(t)
        ps_rs = T(pps, [E, 1], FP32, "ps")
        for i in range(ATTM_T):
            nc.tensor.matmul(ps_rs[:], gmat_t[:, i * E:(i + 1) * E],
                             arsumb_t[:, i:i + 1],
                             start=(i == 0), stop=(i == ATTM_T - 1))
        rsum_t = T(pp, [E, 1], FP32, "rsum")
        nc.vector.tensor_scalar_add(rsum_t[:], ps_rs[:], 1e-5)
        recip_t = T(pp, [E, 1], FP32, "recip")
        nc.vector.reciprocal(recip_t[:], rsum_t[:])
        # rn = rsum/(rsum+eps) = 1 - eps*recip
        # recip as a row vector broadcast across partitions
        ps_rt = T(pps, [1, E], FP32, "ps")
        nc.tensor.transpose(ps_rt[:], recip_t[:], f32p_t[0:E, 22:44])
        recipT_t = T(pp, [1, E], FP32, "recipT")
        nc.vector.tensor_copy(recipT_t[:], ps_rt[:])
        recipT_bc = T(pp, [128, E], FP32, "recipT_bc")
        nc.gpsimd.partition_broadcast(recipT_bc[:], recipT_t[:])

        # eanXT[h,e] tiles directly (no transposes), normalized per entity
        ps_xT = [T(pps, [128, E], FP32, "ps") for _ in range(KT_H)]
        for ct in range(8):
            for ht in range(KT_H):
                nc.tensor.matmul(ps_xT[ht][:],
                                 xfull_t[:, ct * H + ht * 128:ct * H + (ht + 1) * 128],
                                 eaTb_t[ct][:], start=(ct == 0), stop=(ct == 7))
        eanXT_t = []
        for ht in range(KT_H):
            t = T(pp, [128, E], SEQ_DT, f"eanXT{ht}")
            nc.vector.tensor_mul(t[:], ps_xT[ht][:], recipT_bc[:])
            eanXT_t.append(t)
        chaff(6)
        ps_ectxT = [T(pps, [128, E], FP32, "ps") for _ in range(4)]
        for ht in range(KT_H):
            for mt in range(4):
                nc.tensor.matmul(ps_ectxT[mt][:],
                                 wtrans_t[:, ht * EMB + mt * 128:ht * EMB + (mt + 1) * 128],
                                 eanXT_t[ht][:], start=(ht == 0), stop=(ht == KT_H - 1))
        # rnT = rowsum(ean) as a row = 1 - eps*recipT
        rnT_t = T(pp, [1, E], FP32, "rnT")
        nc.vector.tensor_scalar_mul(rnT_t[:], recipT_t[:], -1e-5)
        nc.vector.tensor_scalar_add(rnT_t[:], rnT_t[:], 1.0)
        rnT_bc = T(pp, [128, E], FP32, "rnT_bc")
        nc.gpsimd.partition_broadcast(rnT_bc[:], rnT_t[:])
        ectxT_t = []
        for mt in range(4):
            bt_ = T(pst, [128, E], FP32, "ebias", bufs=2)
            nc.vector.tensor_scalar_mul(bt_[:], rnT_bc[:], f32p_t[:, 70 + mt:71 + mt])
            t = T(pp, [128, E], PAIR_DT, f"ectxT{mt}")
            nc.vector.tensor_add(t[:], ps_ectxT[mt][:], bt_[:])
            ectxT_t.append(t)
        ectxb_t = T(pp, [E, EMB], PAIR_DT, "ectxb")
        for mt in range(4):
            ps = T(pps, [E, 64], FP32, "ps")
            psb = ps[:].bitcast(PAIR_DT)
            nc.tensor.transpose(psb, ectxT_t[mt][:], bf16p_t[:, 0:128])
            nc.vector.tensor_copy(ectxb_t[:, mt * 128:(mt + 1) * 128], psb)

        # ---- S5: RGCN (entity rows only) ----
        ps_gcnT = [T(pps, [128, E], FP32, "ps") for _ in range(4)]
        term = 0
        for r in range(5):
            for i in range(4):
                for mt in range(4):
                    nc.tensor.matmul(
                        ps_gcnT[mt][:],
                        wrel_t[:, (r * 4 + i) * EMB + mt * 128:(r * 4 + i) * EMB + (mt + 1) * 128],
                        msgE_t[i][:, r * E:(r + 1) * E],
                        start=(term == 0), stop=False)
                term += 1
        for r in range(5):
            for mt in range(4):
                nc.tensor.matmul(ps_gcnT[mt][:],
                                 wrels_t[0:20, r * EMB + mt * 128:r * EMB + (mt + 1) * 128],
                                 msgE_t[4][0:20, r * E:(r + 1) * E],
                                 start=False, stop=(r == 4))
        nc.gpsimd.dma_start(w1c_t[3][:, 0:3200], w1[:, 19200:22400])
        nc.gpsimd.dma_start(w1c_t[3][:, 3200:6400], w1[:, 22400:25600])
        gcnT_t = []
        for mt in range(4):
            t = T(pp, [128, E], PAIR_DT, f"gcnT{mt}")
            nc.scalar.activation(t[:], ps_gcnT[mt][:], ACT.Relu,
                                 bias=f32p_t[:, 66 + mt:67 + mt])
            gcnT_t.append(t)
        entb_t = T(pp, [E, EMB], PAIR_DT, "entb")
        for mt in range(4):
            ps = T(pps, [E, 64], FP32, "ps")
            psb = ps[:].bitcast(PAIR_DT)
            nc.tensor.transpose(psb, gcnT_t[mt][:], bf16p_t[:, 0:128])
            nc.vector.tensor_copy(entb_t[:, mt * 128:(mt + 1) * 128], psb)
        featT = [None] * 16
        for srх in range(2):
            for mt in range(4):
                ps = T(pps, [128, PH2], FP32, "ps")
                nc.tensor.matmul(ps[:], entb_t[:, mt * 128:(mt + 1) * 128],
                                 shst_t[:, srх * PH2:(srх + 1) * PH2],
                                 start=True, stop=True)
                t = T(pp, [128, PH2], PAIR_DT, f"featT{4 * srх + mt}")
                nc.vector.tensor_copy(t[:], ps[:])
                featT[4 * srх + mt] = t
        for mt in range(4):
            t = T(pp, [128, PH2], PAIR_DT, f"featT{12 + mt}")
            nc.vector.tensor_mul(t[:], featT[mt][:], featT[4 + mt][:])
            featT[12 + mt] = t

        # ---- S6: relation map x (bf16 transposes + row selection) ----
        ps_c1 = [T(pps, [128, R1 * S], FP32, "ps") for _ in range(2)]
        entT_t, ectxTv_t = gcnT_t, ectxT_t
        entS_t, ectxS_t = [], []
        for mt in range(4):
            for src_t, dst_list, nf in ((entb_t, entS_t, "entS"),
                                        (ectxb_t, ectxS_t, "ectxS")):
                ps = T(pps, [128, 17], FP32, "ps")
                nc.tensor.matmul(ps[:], src_t[:, mt * 128:(mt + 1) * 128], psel_t,
                                 start=True, stop=True)
                t = T(pp, [128, 17], PAIR_DT, f"{nf}{mt}")
                nc.vector.tensor_copy(t[:], ps[:])
                dst_list.append(t)
            t1 = T(pst, [128, 17 * S], FP32, "xtmp", bufs=2)
            nc.vector.tensor_mul(
                t1[:].rearrange("p (a b) -> p a b", a=17, b=S),
                entS_t[mt][:].unsqueeze(2).to_broadcast((128, 17, S)),
                entT_t[mt][:].unsqueeze(1).to_broadcast((128, 17, S)))
            t2 = T(pst, [128, 17 * S], FP32, "xtmp", bufs=2)
            nc.vector.tensor_mul(
                t2[:].rearrange("p (a b) -> p a b", a=17, b=S),
                ectxS_t[mt][:].unsqueeze(2).to_broadcast((128, 17, S)),
                ectxTv_t[mt][:].unsqueeze(1).to_broadcast((128, 17, S)))
            inner = xpad_t[mt][:].rearrange("p (a b) -> p a b", a=BIN, b=PW)[
                :, 2:2 + 17, 2:2 + S]
            nc.vector.tensor_add(inner,
                                 t1[:].rearrange("p (a b) -> p a b", a=17, b=S),
                                 t2[:].rearrange("p (a b) -> p a b", a=17, b=S))
            if mt == 0:
                chaff(16)

        # conv1: 512 -> 256
        for kt in range(4):
            w = w1c_t[kt]
            for tap in range(25):
                di, dj = divmod(tap, 5)
                rhs = xpad_t[kt][:].rearrange("p (a b) -> p a b", a=BIN, b=PW)[
                    :, di:di + R1, dj:dj + S]
                for mt in range(2):
                    nc.tensor.matmul(ps_c1[mt][:],
                                     w[:, tap * 256 + mt * 128:tap * 256 + (mt + 1) * 128],
                                     rhs, start=(kt == 0 and tap == 0),
                                     stop=(kt == 3 and tap == 24))
        for mt in range(2):
            inner = pad1_t[mt][:].rearrange("p (a b) -> p a b", a=B1R, b=PW)[
                :, 2:2 + R1, 2:2 + S]
            nc.scalar.activation(inner,
                                 ps_c1[mt][:].rearrange("p (a b) -> p a b", a=R1, b=S),
                                 ACT.Relu, bias=biasp_t[:, mt:mt + 1])

        # conv2: 256 -> 256
        ps_c2 = [T(pps, [128, R2 * S], FP32, "ps") for _ in range(2)]
        for kt in range(2):
            w = T(pst, [128, 6400], CONV_DT, "wconv", bufs=5)
            dma(w[:, 0:3200], w2[:, kt * 6400:kt * 6400 + 3200])
            dma(w[:, 3200:6400], w2[:, kt * 6400 + 3200:(kt + 1) * 6400])
            for tap in range(25):
                di, dj = divmod(tap, 5)
                rhs = pad1_t[kt][:].rearrange("p (a b) -> p a b", a=B1R, b=PW)[
                    :, di:di + R2, dj:dj + S]
                for mt in range(2):
                    nc.tensor.matmul(ps_c2[mt][:],
                                     w[:, tap * 256 + mt * 128:tap * 256 + (mt + 1) * 128],
                                     rhs, start=(kt == 0 and tap == 0),
                                     stop=(kt == 1 and tap == 24))
        for mt in range(2):
            inner = pad2_t[mt][:].rearrange("p (a b) -> p a b", a=B2R, b=PW)[
                :, 2:2 + R2, 2:2 + S]
            nc.scalar.activation(inner,
                                 ps_c2[mt][:].rearrange("p (a b) -> p a b", a=R2, b=S),
                                 ACT.Relu, bias=biasp_t[:, 2 + mt:3 + mt])

        # wht streamed now so its 4.2MB overlaps conv2/conv3 compute
        whtc = []
        for c in range(2):
            t = T(pst, [128, 8 * 1024], PAIR_DT, "bigreuse", bufs=2)
            dma(t[:], wht[:, c * 8192:(c + 1) * 8192])
            whtc.append(t)

        # retire chaff psum so it isn't dead code
        warm_sb = T(pp, [128, 128], FP32, "warm_sb")
        nc.vector.tensor_copy(warm_sb[:], chaff_ps[:])
        warm_dram = pdram.tile([128, 128], FP32, name="warm_dram")
        dma(warm_dram[:], warm_sb[:])

        # conv3: 256 -> 512, four (kt, taphalf) chunks
        x3_t = [T(pp, [128, SP2], PAIR_DT, f"x3_{mt}") for mt in range(4)]
        ps_c3 = [T(pps, [128, SP2], FP32, "ps") for _ in range(4)]
        for kt in range(2):
            for taps in (range(0, 13), range(13, 25)):
                w = T(pst, [128, len(taps) * EMB], CONV_DT, "wconv", bufs=5)
                dma(w[:], w3[:, (kt * 25 + taps.start) * EMB:(kt * 25 + taps.stop) * EMB])
                for tj, tap in enumerate(taps):
                    di, dj = divmod(tap, 5)
                    rhs = pad2_t[kt][:].rearrange("p (a b) -> p a b", a=B2R, b=PW)[
                        :, di:di + R3, dj:dj + S]
                    for mt in range(4):
                        nc.tensor.matmul(ps_c3[mt][:],
                                         w[:, tj * EMB + mt * 128:tj * EMB + (mt + 1) * 128],
                                         rhs, start=(kt == 0 and tap == 0),
                                         stop=(kt == 1 and tap == 24))
        for mt in range(4):
            nc.scalar.activation(x3_t[mt][:], ps_c3[mt][:], ACT.Relu,
                                 bias=biasp_t[:, 4 + mt:5 + mt])

        # ---- S7: pair features + classifier ----
        SP_TILES = [(0, 128), (128, SP2 - 128)]
        x3T_t = [T(pp, [sz, EMB], PAIR_DT, f"x3T{i}")
                 for i, (off, sz) in enumerate(SP_TILES)]
        for i, (off, sz) in enumerate(SP_TILES):
            for src in range(4):
                ps = T(pps, [sz, 64], FP32, "ps")
                psb = ps[:].bitcast(PAIR_DT)
                nc.tensor.transpose(psb, x3_t[src][:, off:off + sz], bf16p_t[:, 0:128])
                nc.vector.tensor_copy(x3T_t[i][:, src * 128:(src + 1) * 128], psb)
        chaff(20)

        smp_t = T(pp, [128, 2 * PH2], PAIR_DT, "smp")
        dma(smp_t[:], smp)

        for mt in range(4):
            ps = T(pps, [128, PH2], FP32, "ps")
            for i, (off, sz) in enumerate(SP_TILES):
                nc.tensor.matmul(ps[:], x3T_t[i][0:sz, mt * 128:(mt + 1) * 128],
                                 smp_t[0:sz, i * PH2:(i + 1) * PH2],
                                 start=(i == 0), stop=(i == 1))
            t = T(pp, [128, PH2], PAIR_DT, f"featT{8 + mt}")
            nc.vector.tensor_copy(t[:], ps[:])
            featT[8 + mt] = t

        chaff(12)
        ps_ht = [T(pps, [128, PH2], FP32, "ps") for _ in range(8)]
        for kt in range(16):
            wv = whtc[kt // 8]
            for mt in range(8):
                nc.tensor.matmul(
                    ps_ht[mt][:],
                    wv[:, (kt % 8) * 1024 + mt * 128:(kt % 8) * 1024 + (mt + 1) * 128],
                    featT[kt][:], start=(kt == 0), stop=(kt == 15))
        htT_t = []
        for mt in range(8):
            t = T(pp, [128, PH2], PAIR_DT, f"htT{mt}")
            nc.scalar.activation(t[:], ps_ht[mt][:], ACT.Tanh,
                                 bias=biasp_t[:, 12 + mt:13 + mt])
            htT_t.append(t)

        chaff(6)
        wbil_t = T(pp, [128, 8 * 97], PAIR_DT, "wbil")
        dma(wbil_t[:], wbil)
        ps_out = T(pps, [97, PH2], FP32, "ps")
        for kt in range(8):
            nc.tensor.matmul(ps_out[:], wbil_t[:, kt * 97:(kt + 1) * 97],
                             htT_t[kt][:], start=(kt == 0), stop=(kt == 7))
        out_t = T(pp, [97, PH2], FP32, "out")
        nc.vector.tensor_scalar_add(out_t[:], ps_out[:], biasp_t[0:97, 20:21])
        dma(outt, out_t[:])

    nc.compile()
    return nc


_PROG = None


def _get_prog():
    global _PROG
    if _PROG is None:
        _PROG = build_program()
    return _PROG


def _np(dt):
    return _NPDT[dt]


def _pack_rows(a, ntiles, dt):
    """[ntiles*128, W] (zero-padded) -> [128, ntiles*W] with tile i at
    column block i."""
    r, w = a.shape
    pad = ntiles * 128 - r
    if pad:
        a = np.concatenate([a, np.zeros((pad, w), a.dtype)], axis=0)
    return np.ascontiguousarray(
        a.reshape(ntiles, 128, w).transpose(1, 0, 2).reshape(128, ntiles * w),
        _np(dt))


def _pack_conv(w, flip, dt):
    """conv weight OIHW -> tap-major per-kt chunks [128, ...]."""
    w = np.asarray(w, np.float32)
    if flip:
        w = w[:, :, ::-1, :]
    oc, ic, _, _ = w.shape
    t = w.transpose(2, 3, 1, 0).reshape(25, ic, oc)   # (tap, ic, oc)
    nkt = ic // 128
    chunks = [np.ascontiguousarray(
        t[:, kt * 128:(kt + 1) * 128, :].transpose(1, 0, 2).reshape(128, 25 * oc))
        for kt in range(nkt)]
    return np.ascontiguousarray(np.concatenate(chunks, axis=1), _np(dt))


def _pack_conv3(w, flip, dt):
    """conv3 weights as (kt, taphalf) chunks: [128, 25*512] per kt with
    taps in order — column block (kt*25 + tap)*512."""
    w = np.asarray(w, np.float32)
    if flip:
        w = w[:, :, ::-1, :]
    t = w.transpose(2, 3, 1, 0).reshape(25, IC, EMB)
    chunks = [np.ascontiguousarray(
        t[:, kt * 128:(kt + 1) * 128, :].transpose(1, 0, 2).reshape(128, 25 * EMB))
        for kt in range(2)]
    return np.ascontiguousarray(np.concatenate(chunks, axis=1), _np(dt))


def _shared_inputs(inputs):
    f32 = np.float32
    sh = {}
    fp = np.zeros((128, 74), f32)
    fp[:, 0] = np.asarray(inputs["conv1_b"], f32)[0:128]
    fp[:, 1] = np.asarray(inputs["conv1_b"], f32)[128:256]
    fp[:, 2] = np.asarray(inputs["conv2_b"], f32)[0:128]
    fp[:, 3] = np.asarray(inputs["conv2_b"], f32)[128:256]
    for mt in range(4):
        fp[:, 4 + mt] = np.asarray(inputs["conv3_b"], f32)[mt * 128:(mt + 1) * 128]
    for mt in range(8):
        fp[:, 12 + mt] = np.asarray(inputs["ht_b"], f32)[mt * 128:(mt + 1) * 128]
    fp[0:97, 20] = np.asarray(inputs["bil_b"], f32)
    fp[:, 21] = 1.0
    fp[0:22, 22:44] = np.eye(22, dtype=f32)
    fp[0:E * M, 44:66] = np.kron(np.eye(E, dtype=f32), np.ones((M, 1), f32))
    for mt in range(4):
        fp[:, 66 + mt] = np.asarray(inputs["b_rgcn"], f32)[mt * 128:(mt + 1) * 128]
        fp[:, 70 + mt] = np.asarray(inputs["b_trans"], f32)[mt * 128:(mt + 1) * 128]
    sh["f32p"] = fp
    bb = np.zeros((1, 1024), f32)
    bb[0, 0:512] = np.asarray(inputs["b_trans"], f32)
    bb[0, 512:1024] = np.asarray(inputs["b_rgcn"], f32)
    sh["btb"] = bb
    bt = np.zeros((128, 1206), np.float32)
    bt[:, 0:128] = np.eye(128, dtype=f32)
    bt[:, 524:644] = _pack_rows(np.kron(np.eye(L, dtype=f32),
                                        np.ones((LS, 1), f32)), SPAN_T, FP32)
    bt[:, 644:798] = _pack_rows(np.kron(np.eye(E, dtype=f32),
                                        np.ones((M * NH, 1), f32) / (M * NH)),
                                ATTM_T, FP32)
    bt[:, 1205] = 1.0
    sh["bf16p_base"] = bt
    sh["wtrans"] = _pack_rows(np.asarray(inputs["W_trans"], f32), KT_H, SEQ_DT)
    # wrel: 5 relations (4 + self); big k-tiles [128] and the 20-row tail
    wr = np.zeros((5, 532, EMB), f32)
    wr[0:4] = np.asarray(inputs["W_rel"], f32)
    wr[4] = np.asarray(inputs["W_self"], f32)
    wrb = wr[:, 0:512, :].reshape(5, 4, 128, EMB)   # (r, i, 128, 512)
    sh["wrel"] = np.ascontiguousarray(
        wrb.transpose(2, 0, 1, 3).reshape(128, 20 * EMB), _np(GRAPH_DT))
    sh["wrels"] = np.ascontiguousarray(
        wr[:, 512:532, :].transpose(1, 0, 2).reshape(20, 5 * EMB), _np(GRAPH_DT))
    sh["w1"] = [_pack_conv(inputs["conv1_w"], fl, CONV_DT) for fl in (0, 1)]
    sh["w2"] = [_pack_conv(inputs["conv2_w"], fl, CONV_DT) for fl in (0, 1)]
    sh["w3"] = [_pack_conv3(inputs["conv3_w"], fl, CONV_DT) for fl in (0, 1)]
    sh["wht"] = _pack_rows(np.asarray(inputs["ht_W"], f32), 16, PAIR_DT)
    sh["wbil"] = _pack_rows(np.asarray(inputs["bil_W"], f32), 8, PAIR_DT)
    psel = []
    for fl in (0, 1):
        pm = np.zeros((E, 17), f32)
        for r in range(17):
            pm[(21 - r) if fl else r, r] = 1.0
        psel.append(pm)
    sh["psel"] = psel
    return sh


def _pair_idx(hts_b, hh):
    h = np.asarray(hts_b)[:, 0]
    mask = (h <= 10) if hh == 0 else (h >= 11)
    idx = np.nonzero(mask)[0]
    if len(idx) > PH2:
        raise RuntimeError(f"pair overflow: {len(idx)} > {PH2}")
    return idx


def _core_inputs(inputs, shared, b, hh):
    f32 = np.float32
    X = np.asarray(inputs["sequence_output"][b], f32)
    att = np.asarray(inputs["attention"][b], f32)
    adj = np.asarray(inputs["adjacency"][b], f32)
    mf = np.asarray(inputs["mention_idx"][b]).reshape(-1).astype(np.int64)
    ls = np.asarray(inputs["link_start"][b]).reshape(-1).astype(np.int64)
    ntypes = np.asarray(inputs["node_types"][b]).astype(np.int64)
    hts = np.asarray(inputs["hts"][b]).astype(np.int64)

    m = {k: shared[k] for k in
         ("f32p", "btb", "wtrans", "wrel", "wrels", "wht", "wbil")}
    m["w1"] = shared["w1"][hh]
    m["w2"] = shared["w2"][hh]
    m["w3"] = shared["w3"][hh]
    bt = shared["bf16p_base"].copy()
    bt[:, 128:524] = _pack_rows(np.ascontiguousarray(X[mf].T), KT_H, FP32)
    bt[0:E, 798:815] = shared["psel"][hh]
    te = np.asarray(inputs["type_embed"], f32)[ntypes]
    adjc = np.concatenate([adj[r].T[:, 0:E] for r in range(4)]
                          + [np.eye(NN, E, dtype=f32)], axis=1)
    for gi, (goff, gsz) in enumerate([(0, E), (E, E * M), (E + E * M, L)]):
        bt[0:gsz, 815 + gi * 20:835 + gi * 20] = te[goff:goff + gsz]
        bt[0:gsz, 875 + gi * 110:985 + gi * 110] = adjc[goff:goff + gsz]
    m["bf16p"] = np.ascontiguousarray(bt, _np(PAIR_DT))
    pos = ls[:, None] + np.arange(LS)
    m["xspan"] = _pack_rows(X[pos.reshape(-1)], SPAN_T, SEQ_DT)
    m["xfull"] = _pack_rows(X, 8, SEQ_DT)
    rows = att[:, mf, :]
    m["attm"] = _pack_rows(rows.transpose(1, 0, 2).reshape(E * M * NH, C),
                           ATTM_T, GRAPH_DT)
    attl = np.empty((L * LS, NH * LS), f32)
    for l in range(L):
        blk = att[:, pos[l], :][:, :, pos[l]]
        attl[l * LS:(l + 1) * LS, :] = blk.transpose(2, 0, 1).reshape(LS, NH * LS)
    m["attl"] = _pack_rows(attl, SPAN_T, GRAPH_DT)

    idx = _pair_idx(hts, hh)
    pr = hts[idx]
    n = len(idx)
    shm = np.zeros((E, 2 * PH2), f32)
    shm[pr[:, 0], np.arange(n)] = 1.0
    shm[pr[:, 1], PH2 + np.arange(n)] = 1.0
    m["shst"] = np.ascontiguousarray(shm, _np(PAIR_DT))
    loc_r = pr[:, 0] if hh == 0 else (21 - pr[:, 0])
    smm = np.zeros((SP2, PH2), f32)
    smm[loc_r * S + pr[:, 1], np.arange(n)] = 1.0
    m["smp"] = _pack_rows(smm, 2, PAIR_DT)
    return m


def kernel(**inputs):
    nc = _get_prog()
    shared = _shared_inputs(inputs)
    in_maps = []
    for b in range(B):
        for hh in range(2):
            in_maps.append(_core_inputs(inputs, shared, b, hh))
    res = run_bass_kernel_spmd(nc, in_maps, core_ids=list(range(8)))
    out = np.empty((B, P, 97), np.float32)
    for b in range(B):
        for hh in range(2):
            idx = _pair_idx(np.asarray(inputs["hts"][b]), hh)
            r = np.asarray(res.results[2 * b + hh]["outt"], np.float32)
            out[b, idx, :] = r[:, 0:len(idx)].T
    return out
